# revision 12
# baseline (speedup 1.0000x reference)
"""Trainium2 Bass kernel for a 3-layer Lorentz (hyperboloid) MLP.

Math: the reference chains lorentz_linear + inter-layer projx(expmap0(logmap0(.))).
Algebraically, expmap0 -> projx -> logmap0 round-trips cancel: the inter-layer op
on the tangent vector y is exactly "zero the time component, clamp the row norm
of y[1:] to 10".  So the network is:

  t0 = logmap0(x)                       (row scale d/||xs|| on xs, time comp 0)
  y1 = t0 @ W1.T + b1 ; t1 = clamp(y1)  (zero col 0, clamp row norm to 10)
  y2 = t1 @ W2.T + b2 ; t2 = clamp(y2)
  y3 = t2 @ W3.T + b3
  out = [cosh(nc), sinh(nc)/n * y3[1:]] with n=clip(||y3[1:]||,eps), nc=min(n,10)

With zero biases (the shipped case), per-token scales commute through the
GEMMs; logmap0 is applied on the host (t0 shipped pre-scaled in bf16) and the
inter-layer clamps are folded into a cumulative SQUARED per-token scale cs2
tracked on [1,tok] vectors, applied once at the very end.  The PE runs the
three GEMMs back to back with no inter-layer barrier.

All scalar-engine transcendentals (Ln, Exp, Identity, Square) live in the
single `natural_log_exp_and_others` table set, so there are no mid-kernel
ACT_TABLE_LOADs; rsqrt/sqrt are computed as exp(+-0.5*ln(x)) which is also
far more accurate than the Sqrt table (65536-ULP budget).

Layout: everything on-chip is FEATURE-major ([feat, token]); weights are
pre-transposed/blocked/bf16-cast on the host so each m-tile loads with one
fully contiguous DMA.  Row-wise (per-token) sum-of-squares are ones-vector
matmuls on the TensorEngine (partition-dim reduction), deferred one m-tile
into the next layer's PE stream so the PE never stalls on them.  GpSimd does
nothing but the two final partition_broadcasts (single ucode lib, loaded once
during the MM stream).

Sharding: pure data-parallel over tokens - 8192 tokens -> 8 cores x 1024.

With nonzero biases a general barrier path (scale applied between layers,
arccosh on device) is kept as fallback.
"""

import math
import os
import sys
import functools

import numpy as np
import ml_dtypes


def _import_concourse():
    try:
        import concourse  # noqa: F401
    except ImportError:
        for p in ("/opt/trn_rl_repo", "/root/.axon_site/_ro/trn_rl_repo"):
            if os.path.isdir(p) and p not in sys.path:
                sys.path.insert(0, p)
        import concourse  # noqa: F401


_import_concourse()

import concourse.bass as bass  # noqa: E402,F401
import concourse.bacc as bacc  # noqa: E402
import concourse.mybir as mybir  # noqa: E402
import concourse.tile as tile  # noqa: E402
from concourse import bass_utils  # noqa: E402

F32 = mybir.dt.float32
BF16 = mybir.dt.bfloat16
AF = mybir.ActivationFunctionType
ALU = mybir.AluOpType

P = 128
N_CORES = 8
EPS = 1e-7
EPS2 = 1e-14  # EPS**2, the ln(ssq + eps^2) guard
MAX_TAN_NORM = 10.0
LN10 = math.log(10.0)
LNHALF = math.log(0.5)

# Full-problem dims (hardcoded per spec)
TOK, D_IN, D_HID, D_OUT = 8192, 1024, 4096, 1024
TOKPC = TOK // N_CORES  # tokens per core


# =====================================================================
# Fold-mode program (zero biases - the shipped case)
# =====================================================================

def build_nc_fold(tokpc=TOKPC, din=D_IN, dhid=D_HID, dout=D_OUT, ch=512):
    assert tokpc % ch == 0
    nch = tokpc // ch
    kt1, mt1 = din // P, dhid // P
    kt2, mt2 = dhid // P, dhid // P
    kt3, mt3 = dhid // P, dout // P

    nc = bacc.Bacc("TRN2", target_bir_lowering=False, debug=False,
                   num_devices=N_CORES)

    xt_d = nc.dram_tensor("xt", [din, tokpc], BF16, kind="ExternalInput")
    w1_d = nc.dram_tensor("w1", [mt1 * P, din], BF16, kind="ExternalInput")
    w2_d = nc.dram_tensor("w2", [mt2 * P, dhid], BF16, kind="ExternalInput")
    w3_d = nc.dram_tensor("w3", [mt3 * P, dhid], BF16, kind="ExternalInput")
    out_d = nc.dram_tensor("out", [dout, tokpc], F32, kind="ExternalOutput")

    with tile.TileContext(nc) as tc:
        _build_fold_program(tc, nc, dict(
            tokpc=tokpc, ch=ch, nch=nch,
            kt1=kt1, mt1=mt1, kt2=kt2, mt2=mt2, kt3=kt3, mt3=mt3,
            xt=xt_d, w1=w1_d, w2=w2_d, w3=w3_d, out=out_d,
        ))
    nc.compile()
    return nc


def _build_fold_program(tc, nc, C):
    tokpc, ch, nch = C["tokpc"], C["ch"], C["nch"]

    const = tc.alloc_tile_pool(name="const", bufs=1)
    scalL = tc.alloc_tile_pool(name="scalL", bufs=2)   # [1, tokpc] f32 (cs2)
    scalS = tc.alloc_tile_pool(name="scalS", bufs=6)   # [1, ch] f32
    bcast = tc.alloc_tile_pool(name="bcast", bufs=2)
    sqp = tc.alloc_tile_pool(name="sq", bufs=2)
    accp = tc.alloc_tile_pool(name="acc", bufs=4)
    wp = tc.alloc_tile_pool(name="wt", bufs=3)
    psy = tc.alloc_tile_pool(name="psy", bufs=6, space="PSUM")
    psn = tc.alloc_tile_pool(name="psn", bufs=2, space="PSUM")
    outp = tc.alloc_tile_pool(name="outp", bufs=4)

    ones_f = const.tile([P, 1], F32, tag="ones_f", name="ones_f")
    nc.vector.memset(ones_f[:], 1.0)
    c_eps2 = const.tile([P, 1], F32, tag="c_eps2", name="c_eps2")
    nc.vector.memset(c_eps2[:], EPS2)
    c_ln10 = const.tile([P, 1], F32, tag="c_ln10", name="c_ln10")
    nc.vector.memset(c_ln10[:], LN10)
    c_lnhalf = const.tile([P, 1], F32, tag="c_lnhalf", name="c_lnhalf")
    nc.vector.memset(c_lnhalf[:], LNHALF)

    # ---- head: first GEMM's gating DMAs go out first, split into
    # partition-halves for DMA-ring parallelism ----
    t0p = tc.alloc_tile_pool(name="t0", bufs=1, side="right")
    w1_pre = []
    t0 = []

    wm0 = wp.tile([P, C["kt1"] * P], BF16, tag="wtile", name="wm0")
    t = t0p.tile([P, tokpc], BF16, tag="t0_0", name="t0_0")
    H = P // 2
    nc.sync.dma_start(wm0[0:H, :], C["w1"].ap()[0:H, :])
    nc.sync.dma_start(t[0:H, :], C["xt"].ap()[0:H, :])
    nc.sync.dma_start(wm0[H:P, :], C["w1"].ap()[H:P, :])
    nc.sync.dma_start(t[H:P, :], C["xt"].ap()[H:P, :])
    w1_pre.append(wm0)
    t0.append(t)
    wm1 = wp.tile([P, C["kt1"] * P], BF16, tag="wtile", name="wm1")
    nc.sync.dma_start(wm1[:], C["w1"].ap()[P:2 * P, :])
    w1_pre.append(wm1)
    for k in range(1, C["kt1"]):
        t = t0p.tile([P, tokpc], BF16, tag=f"t0_{k}", name=f"t0_{k}")
        nc.sync.dma_start(t[:], C["xt"].ap()[k * P:(k + 1) * P, :])
        t0.append(t)

    # ---- HAM warmup: ~10 dummy matmuls during the DMA window get the PE
    # clock-gate to K=8/8 before the first real matmul arrives ----
    warm_src = const.tile([P, ch], BF16, tag="warm", name="warm")
    nc.vector.memset(warm_src[:], 1.0)
    ps_warm = psn.tile([1, ch], F32, tag="psn", name="ps_warm")
    for _ in range(10):
        nc.tensor.matmul(ps_warm[:], warm_src[:, 0:1], warm_src[:],
                         start=True, stop=True)

    def stile_s():
        return scalS.tile([1, ch], F32, tag="ss", name="ss")

    S = {"cs2": None}

    def gemm_layer(tin, w_d, kt, mt, out_pool, tag, mid_fn=None,
                   preloaded=()):
        """y[m] = sum_k w[k,m].T @ tin[k]; ACT evicts and squares straight
        from PSUM; squares accumulate on the DVE (f32) and a single fp32
        ones-matmul per chunk (deferred via finish()) does the final
        partition-reduce.  mid_fn is emitted after m==1's matmuls."""
        accs = [accp.tile([P, ch], F32, tag="acc", name=f"acc{_}")
                for _ in range(nch)]
        tout = []
        for m in range(mt):
            if m < len(preloaded):
                wm = preloaded[m]
            else:
                wm = wp.tile([P, kt * P], BF16, tag="wtile", name="wm")
                nc.sync.dma_start(wm[:], w_d.ap()[m * P:(m + 1) * P, :])
            pss = [psy.tile([P, ch], F32, tag="psy", name=f"psy{_}")
                   for _ in range(nch)]
            for k in range(kt):
                for c in range(nch):
                    nc.tensor.matmul(pss[c][:], wm[:, k * P:(k + 1) * P],
                                     tin[k][:, c * ch:(c + 1) * ch],
                                     start=(k == 0), stop=(k == kt - 1))
            if m == 1 and mid_fn is not None:
                mid_fn()
            ty = out_pool.tile([P, tokpc], BF16, tag=f"{tag}{m}",
                               name=f"{tag}{m}")
            for c in range(nch):
                if m == 0:
                    nc.scalar.activation(accs[c][:], pss[c][:], AF.Square)
                    nc.vector.memset(accs[c][0:1, :], 0.0)
                else:
                    sq = sqp.tile([P, ch], F32, tag="sq", name="sq")
                    nc.scalar.activation(sq[:], pss[c][:], AF.Square)
                    nc.vector.tensor_tensor(accs[c][:], accs[c][:], sq[:],
                                            ALU.add)
                nc.scalar.activation(ty[:, c * ch:(c + 1) * ch], pss[c][:],
                                     AF.Identity)
            if m == 0:
                nc.vector.memset(ty[0:1, :], 0.0)
            tout.append(ty)

        def finish():
            ps_norm = [psn.tile([1, ch], F32, tag="psn", name=f"psn{_}")
                       for _ in range(nch)]
            for c in range(nch):
                nc.tensor.matmul(ps_norm[c][:], ones_f[:], accs[c][:],
                                 start=True, stop=True)
            return ps_norm
        return tout, finish

    def boundary(fin, first):
        """Inter-layer clamp folded into cs2 (squared cumulative scale).
        f = min(1, 10/sqrt(cs2_prev*ssq_raw)); cs2_new = cs2_prev * f^2.
        rsqrt via exp(-0.5*ln(.)) - Ln pass then Exp pass so the act-table
        set switches at most twice, not per chunk."""
        ps_norm = fin()
        cs2_new = scalL.tile([1, tokpc], F32, tag="cs2", name="cs2")
        Ls = []
        for c in range(nch):
            sl = slice(c * ch, (c + 1) * ch)
            if first:
                src = ps_norm[c][:]
            else:
                m2 = stile_s()
                nc.vector.tensor_tensor(m2[:], ps_norm[c][:],
                                        S["cs2"][:, sl], ALU.mult)
                src = m2[:]
            L = stile_s()
            nc.scalar.activation(L[:], src, AF.Ln, bias=c_eps2[0:1, :])
            Ls.append(L)
        for c in range(nch):
            sl = slice(c * ch, (c + 1) * ch)
            r10 = stile_s()
            nc.scalar.activation(r10[:], Ls[c][:], AF.Exp, scale=-0.5,
                                 bias=c_ln10[0:1, :])
            f = stile_s()
            nc.vector.tensor_scalar_min(f[:], r10[:], 1.0)
            if first:
                nc.vector.tensor_tensor(cs2_new[:, sl], f[:], f[:], ALU.mult)
            else:
                ff = stile_s()
                nc.vector.tensor_tensor(ff[:], f[:], f[:], ALU.mult)
                nc.vector.tensor_tensor(cs2_new[:, sl], S["cs2"][:, sl],
                                        ff[:], ALU.mult)
        S["cs2"] = cs2_new

    # ---------- Layers 1, 2 ----------
    t1p = tc.alloc_tile_pool(name="t1", bufs=1, side="left")
    t1, fin1 = gemm_layer(t0, C["w1"], C["kt1"], C["mt1"], t1p, "t1_",
                          preloaded=w1_pre)
    t0p.release()

    t2p = tc.alloc_tile_pool(name="t2", bufs=1, side="right")
    t2, fin2 = gemm_layer(t1, C["w2"], C["kt2"], C["mt2"], t2p, "t2_",
                          mid_fn=lambda: boundary(fin1, first=True))
    t1p.release()

    # ---------- Layer 3 + expmap0/projx, uneven chunks [512, 256, 256] so
    # the only exposed (last) tail is narrow; earlier tails overlap the next
    # chunk's matmuls ----------
    kt, mt = C["kt3"], C["mt3"]
    l3_chunks = []
    st = 0
    for w in [ch] * nch:
        l3_chunks.append((st, w))
        st += w
    assert st == tokpc
    y3p = tc.alloc_tile_pool(name="y3", bufs=1, side="left")
    y3 = [y3p.tile([P, tokpc], F32, tag=f"y3_{m}", name=f"y3_{m}")
          for m in range(mt)]
    deferred_tail = None
    for ci, (st, w) in enumerate(l3_chunks):
        sl = slice(st, st + w)
        acc3 = accp.tile([P, w], F32, tag="acc", name=f"acc3_{ci}")
        for m in range(mt):
            wm = wp.tile([P, kt * P], BF16, tag="wtile", name="wm")
            nc.sync.dma_start(wm[:], C["w3"].ap()[m * P:(m + 1) * P, :])
            ps = psy.tile([P, w], F32, tag="psy", name="psy3")
            for k in range(kt):
                nc.tensor.matmul(ps[:], wm[:, k * P:(k + 1) * P],
                                 t2[k][:, sl],
                                 start=(k == 0), stop=(k == kt - 1))
            if m == 1:
                if ci == 0:
                    boundary(fin2, first=False)
                    # cs = sqrt(cs2) via exp(0.5*ln(.)), off the critical
                    # path - the final tails then need only ONE Ln each
                    cs = scalL.tile([1, tokpc], F32, tag="cs", name="cs")
                    for cc in range(nch):
                        ssl = slice(cc * ch, (cc + 1) * ch)
                        Lc = stile_s()
                        nc.scalar.activation(Lc[:], S["cs2"][:, ssl], AF.Ln)
                        nc.scalar.activation(cs[:, ssl], Lc[:], AF.Exp,
                                             scale=0.5)
                    S["cs"] = cs
                if deferred_tail is not None:
                    deferred_tail()
                    deferred_tail = None
            if m == 0:
                nc.scalar.activation(acc3[:], ps[:], AF.Square)
                nc.vector.memset(acc3[0:1, :], 0.0)
            else:
                sq = sqp.tile([P, w], F32, tag="sq", name="sq")
                nc.scalar.activation(sq[:], ps[:], AF.Square)
                nc.vector.tensor_tensor(acc3[:], acc3[:], sq[:], ALU.add)
            nc.scalar.activation(y3[m][:, sl], ps[:], AF.Identity)

        def chunk_tail(sl=sl, w=w, acc3=acc3, ci=ci):
            # true norm n = sqrt(cs2*ssq_raw); nc = min(n, 10)
            # out[0] = cosh(nc) = 0.5 e^nc + 0.5 e^-nc
            # out[1:] = y3_raw * sinh(nc) / sqrt(ssq_raw)
            # Ln calls grouped before Exp calls: 2 act-table switches total.
            def stile_w():
                return scalS.tile([1, w], F32, tag="ss", name="ss")
            ps_norm = psn.tile([1, w], F32, tag="psn", name=f"psn3_{ci}")
            nc.tensor.matmul(ps_norm[:], ones_f[:], acc3[:],
                             start=True, stop=True)
            m2 = stile_w()
            nc.vector.tensor_tensor(m2[:], ps_norm[:], S["cs2"][:, sl],
                                    ALU.mult)
            Lm = stile_w()
            nc.scalar.activation(Lm[:], m2[:], AF.Ln, bias=c_eps2[0:1, :])
            inv_n = stile_w()
            nc.scalar.activation(inv_n[:], Lm[:], AF.Exp, scale=-0.5)
            n_ = stile_w()
            nc.vector.tensor_tensor(n_[:], m2[:], inv_n[:], ALU.mult)
            ncl = stile_w()
            nc.vector.tensor_scalar_min(ncl[:], n_[:], MAX_TAN_NORM)
            e_ = stile_w()
            nc.scalar.activation(e_[:], ncl[:], AF.Exp, scale=1.0,
                                 bias=c_lnhalf[0:1, :])
            en = stile_w()
            nc.scalar.activation(en[:], ncl[:], AF.Exp, scale=-1.0,
                                 bias=c_lnhalf[0:1, :])
            cosh_c = stile_w()
            nc.vector.tensor_tensor(cosh_c[:], e_[:], en[:], ALU.add)
            sh = stile_w()
            nc.vector.tensor_tensor(sh[:], e_[:], en[:], ALU.subtract)
            s3 = stile_w()
            nc.vector.tensor_tensor(s3[:], sh[:], inv_n[:], ALU.mult)
            nc.vector.tensor_tensor(s3[:], s3[:], S["cs"][:, sl], ALU.mult)
            s3b = bcast.tile([P, w], F32, tag="s3b", name="s3b")
            nc.gpsimd.partition_broadcast(s3b[:], s3[:])
            last = ci == len(l3_chunks) - 1
            for m in range(mt):
                ot = outp.tile([P, w], F32, tag="ot", name="ot")
                # on the final (exposed) tail, offload some multiplies to
                # GpSimd - no partition_broadcast follows, so the ucode
                # lib swap costs nothing afterwards
                eng = nc.gpsimd if (last and m % 3 == 2) else nc.vector
                eng.tensor_tensor(ot[:], y3[m][:, sl], s3b[:], ALU.mult)
                if m == 0:
                    nc.vector.tensor_copy(ot[0:1, :], cosh_c[:])
                nc.sync.dma_start(C["out"].ap()[m * P:(m + 1) * P, sl],
                                  ot[:])

        deferred_tail = chunk_tail
    deferred_tail()
    t2p.release()
    y3p.release()

    for p in (outp, psn, psy, wp, accp, sqp, bcast, scalS, scalL, const):
        p.release()


# =====================================================================
# General (nonzero-bias) fallback program - barrier between layers
# =====================================================================

def build_nc_general(tokpc=TOKPC, din=D_IN, dhid=D_HID, dout=D_OUT, ch=512):
    assert tokpc % ch == 0
    nch = tokpc // ch
    kt1, mt1 = din // P, dhid // P
    kt2, mt2 = dhid // P, dhid // P
    kt3, mt3 = dhid // P, dout // P

    nc = bacc.Bacc("TRN2", target_bir_lowering=False, debug=False,
                   num_devices=N_CORES)

    xt_d = nc.dram_tensor("xt", [din, tokpc], BF16, kind="ExternalInput")
    x0_d = nc.dram_tensor("x0", [1, tokpc], F32, kind="ExternalInput")
    w1_d = nc.dram_tensor("w1", [mt1 * P, din], BF16, kind="ExternalInput")
    w2_d = nc.dram_tensor("w2", [mt2 * P, dhid], BF16, kind="ExternalInput")
    w3_d = nc.dram_tensor("w3", [mt3 * P, dhid], BF16, kind="ExternalInput")
    b1_d = nc.dram_tensor("b1", [P, mt1], F32, kind="ExternalInput")
    b2_d = nc.dram_tensor("b2", [P, mt2], F32, kind="ExternalInput")
    b3_d = nc.dram_tensor("b3", [P, mt3], F32, kind="ExternalInput")
    out_d = nc.dram_tensor("out", [dout, tokpc], F32, kind="ExternalOutput")

    with tile.TileContext(nc) as tc:
        _build_general_program(tc, nc, dict(
            tokpc=tokpc, din=din, dhid=dhid, dout=dout, ch=ch, nch=nch,
            kt1=kt1, mt1=mt1, kt2=kt2, mt2=mt2, kt3=kt3, mt3=mt3,
            xt=xt_d, x0=x0_d, w1=w1_d, w2=w2_d, w3=w3_d,
            b1=b1_d, b2=b2_d, b3=b3_d, out=out_d,
        ))
    nc.compile()
    return nc


def _build_general_program(tc, nc, C):
    tokpc, ch, nch = C["tokpc"], C["ch"], C["nch"]

    const = tc.alloc_tile_pool(name="const", bufs=1)
    scalL = tc.alloc_tile_pool(name="scalL", bufs=5)
    scalS = tc.alloc_tile_pool(name="scalS", bufs=6)
    bcast = tc.alloc_tile_pool(name="bcast", bufs=2)
    sqp = tc.alloc_tile_pool(name="sq", bufs=2)
    accp = tc.alloc_tile_pool(name="acc", bufs=4)
    wp = tc.alloc_tile_pool(name="wt", bufs=3)
    psy = tc.alloc_tile_pool(name="psy", bufs=4, space="PSUM")
    psn = tc.alloc_tile_pool(name="psn", bufs=4, space="PSUM")
    outp = tc.alloc_tile_pool(name="outp", bufs=4)

    bias1 = const.tile([P, C["mt1"]], F32, tag="bias1")
    nc.sync.dma_start(bias1[:], C["b1"].ap())
    bias2 = const.tile([P, C["mt2"]], F32, tag="bias2")
    nc.sync.dma_start(bias2[:], C["b2"].ap())
    bias3 = const.tile([P, C["mt3"]], F32, tag="bias3")
    nc.sync.dma_start(bias3[:], C["b3"].ap())
    ones_f = const.tile([P, 1], F32, tag="ones_f", name="ones_f")
    nc.vector.memset(ones_f[:], 1.0)

    def stile_l():
        return scalL.tile([1, tokpc], F32, tag="sl", name="sl")

    def stile_s():
        return scalS.tile([1, ch], F32, tag="ss", name="ss")

    def norm_accum_tiles():
        return [psn.tile([1, ch], F32, tag="psn", name=f"psn{_}")
                for _ in range(nch)]

    def bcast_full(s_full):
        sb = bcast.tile([P, tokpc], F32, tag="sb", name="sb")
        nc.gpsimd.partition_broadcast(sb[:], s_full[:])
        return sb

    def gemm_layer(tin, w_d, bias_t, kt, mt, out_pool, out_dtype, tag):
        accs = [accp.tile([P, ch], F32, tag="acc", name=f"acc{_}")
                for _ in range(nch)]
        tout = []
        for m in range(mt):
            wm = wp.tile([P, kt * P], BF16, tag="wtile", name="wm")
            nc.sync.dma_start(wm[:], w_d.ap()[m * P:(m + 1) * P, :])
            pss = [psy.tile([P, ch], F32, tag="psy", name=f"psy{_}")
                   for _ in range(nch)]
            for k in range(kt):
                for c in range(nch):
                    nc.tensor.matmul(pss[c][:], wm[:, k * P:(k + 1) * P],
                                     tin[k][:, c * ch:(c + 1) * ch],
                                     start=(k == 0), stop=(k == kt - 1))
            ty = out_pool.tile([P, tokpc], out_dtype, tag=f"{tag}{m}",
                               name=f"{tag}{m}")
            for c in range(nch):
                nc.scalar.activation(ty[:, c * ch:(c + 1) * ch], pss[c][:],
                                     AF.Identity, bias=bias_t[:, m:m + 1],
                                     scale=1.0)
                if m == 0:
                    nc.scalar.activation(accs[c][:], pss[c][:], AF.Square,
                                         bias=bias_t[:, m:m + 1], scale=1.0)
                    nc.vector.memset(accs[c][0:1, :], 0.0)
                else:
                    sq = sqp.tile([P, ch], F32, tag="sq", name="sq")
                    nc.scalar.activation(sq[:], pss[c][:], AF.Square,
                                         bias=bias_t[:, m:m + 1], scale=1.0)
                    nc.vector.tensor_tensor(accs[c][:], accs[c][:], sq[:],
                                            ALU.add)
            if m == 0:
                nc.vector.memset(ty[0:1, :], 0.0)
            tout.append(ty)

        def finish():
            ps_norm = norm_accum_tiles()
            for c in range(nch):
                nc.tensor.matmul(ps_norm[c][:], ones_f[:], accs[c][:],
                                 start=True, stop=True)
            return ps_norm
        return tout, finish

    def clamp_scale(ps_norm):
        """s = min(max(sqrt(ssq),eps),10)/max(sqrt(ssq),eps) via ln/exp."""
        s = stile_l()
        for c in range(nch):
            L = stile_s()
            nc.scalar.activation(L[:], ps_norm[c][:], AF.Ln, bias=EPS2)
            r10 = stile_s()
            nc.scalar.activation(r10[:], L[:], AF.Exp, scale=-0.5, bias=LN10)
            f = stile_s()
            nc.vector.tensor_scalar_min(f[:], r10[:], 1.0)
            nc.vector.tensor_copy(s[:, c * ch:(c + 1) * ch], f[:])
        return s

    def apply_scale(tiles, sb):
        for t in tiles:
            nc.vector.tensor_tensor(t[:], t[:], sb[:], ALU.mult)

    def body():
        # Phase 0: load bf16 xs (= raw t0), input norm, s0
        t0p = tc.alloc_tile_pool(name="t0", bufs=1, side="right")
        t0 = []
        for k in range(C["kt1"]):
            t = t0p.tile([P, tokpc], BF16, tag=f"t0_{k}", name=f"t0_{k}")
            nc.sync.dma_start(t[:], C["xt"].ap()[k * P:(k + 1) * P, :])
            t0.append(t)
        x0t = stile_l()
        nc.sync.dma_start(x0t[:], C["x0"].ap())

        acc0 = [accp.tile([P, ch], F32, tag="acc", name=f"acc0_{_}")
                for _ in range(nch)]
        for k in range(C["kt1"]):
            for c in range(nch):
                if k == 0:
                    nc.scalar.activation(acc0[c][:],
                                         t0[k][:, c * ch:(c + 1) * ch],
                                         AF.Square)
                else:
                    sq = sqp.tile([P, ch], F32, tag="sq", name="sq")
                    nc.scalar.activation(sq[:], t0[k][:, c * ch:(c + 1) * ch],
                                         AF.Square)
                    nc.vector.tensor_tensor(acc0[c][:], acc0[c][:], sq[:],
                                            ALU.add)

        # norm-MM for the input + s0 = arccosh(x0) / ||xs|| chain
        # (x0 input here is pre-clipped arccosh distance d, computed on host)
        ps_n0 = norm_accum_tiles()
        for c in range(nch):
            nc.tensor.matmul(ps_n0[c][:], ones_f[:], acc0[c][:],
                             start=True, stop=True)
        s0 = stile_l()
        for c in range(nch):
            L = stile_s()
            nc.scalar.activation(L[:], ps_n0[c][:], AF.Ln, bias=EPS2)
            r = stile_s()
            nc.scalar.activation(r[:], L[:], AF.Exp, scale=-0.5)
            nc.vector.tensor_tensor(s0[:, c * ch:(c + 1) * ch],
                                    x0t[:, c * ch:(c + 1) * ch], r[:],
                                    ALU.mult)

        s0b = bcast_full(s0)
        for k in range(C["kt1"]):
            nc.vector.tensor_tensor(t0[k][:], t0[k][:], s0b[:], ALU.mult)

        # Layers 1, 2 with barrier scale application
        t1p = tc.alloc_tile_pool(name="t1", bufs=1, side="left")
        t1, fin1 = gemm_layer(t0, C["w1"], bias1, C["kt1"], C["mt1"],
                              t1p, BF16, "t1_")
        apply_scale(t1, bcast_full(clamp_scale(fin1())))
        t0p.release()

        t2p = tc.alloc_tile_pool(name="t2", bufs=1, side="right")
        t2, fin2 = gemm_layer(t1, C["w2"], bias2, C["kt2"], C["mt2"],
                              t2p, BF16, "t2_")
        apply_scale(t2, bcast_full(clamp_scale(fin2())))
        t1p.release()

        # Layer 3 + expmap0/projx
        kt, mt = C["kt3"], C["mt3"]
        y3p = tc.alloc_tile_pool(name="y3", bufs=1, side="left")
        y3 = [y3p.tile([P, tokpc], F32, tag=f"y3_{m}", name=f"y3_{m}")
              for m in range(mt)]
        deferred_tail = None
        for c in range(nch):
            sl = slice(c * ch, (c + 1) * ch)
            acc3 = accp.tile([P, ch], F32, tag="acc", name=f"acc3_{c}")
            for m in range(mt):
                wm = wp.tile([P, kt * P], BF16, tag="wtile", name="wm")
                nc.sync.dma_start(wm[:], C["w3"].ap()[m * P:(m + 1) * P, :])
                ps = psy.tile([P, ch], F32, tag="psy", name="psy3")
                for k in range(kt):
                    nc.tensor.matmul(ps[:], wm[:, k * P:(k + 1) * P],
                                     t2[k][:, sl],
                                     start=(k == 0), stop=(k == kt - 1))
                if m == 1 and deferred_tail is not None:
                    deferred_tail()
                    deferred_tail = None
                nc.scalar.activation(y3[m][:, sl], ps[:], AF.Identity,
                                     bias=bias3[:, m:m + 1], scale=1.0)
                if m == 0:
                    nc.scalar.activation(acc3[:], ps[:], AF.Square,
                                         bias=bias3[:, m:m + 1], scale=1.0)
                    nc.vector.memset(acc3[0:1, :], 0.0)
                else:
                    sq = sqp.tile([P, ch], F32, tag="sq", name="sq")
                    nc.scalar.activation(sq[:], ps[:], AF.Square,
                                         bias=bias3[:, m:m + 1], scale=1.0)
                    nc.vector.tensor_tensor(acc3[:], acc3[:], sq[:], ALU.add)

            def chunk_tail(sl=sl, acc3=acc3, c=c):
                ps_norm = psn.tile([1, ch], F32, tag="psn", name=f"psn3_{c}")
                nc.tensor.matmul(ps_norm[:], ones_f[:], acc3[:],
                                 start=True, stop=True)
                Lr = stile_s()
                nc.scalar.activation(Lr[:], ps_norm[:], AF.Ln, bias=EPS2)
                rr = stile_s()
                nc.scalar.activation(rr[:], Lr[:], AF.Exp, scale=-0.5)
                n_ = stile_s()
                nc.scalar.activation(n_[:], Lr[:], AF.Exp, scale=0.5)
                ncl = stile_s()
                nc.vector.tensor_scalar_min(ncl[:], n_[:], MAX_TAN_NORM)
                e_ = stile_s()
                nc.scalar.activation(e_[:], ncl[:], AF.Exp, scale=1.0,
                                     bias=LNHALF)
                en = stile_s()
                nc.scalar.activation(en[:], ncl[:], AF.Exp, scale=-1.0,
                                     bias=LNHALF)
                cosh_c = stile_s()
                nc.vector.tensor_tensor(cosh_c[:], e_[:], en[:], ALU.add)
                sh = stile_s()
                nc.vector.tensor_tensor(sh[:], e_[:], en[:], ALU.subtract)
                s3 = stile_s()
                nc.vector.tensor_tensor(s3[:], sh[:], rr[:], ALU.mult)
                s3b = bcast.tile([P, ch], F32, tag="s3b", name="s3b")
                nc.gpsimd.partition_broadcast(s3b[:], s3[:])
                for m in range(mt):
                    ot = outp.tile([P, ch], F32, tag="ot", name="ot")
                    nc.vector.tensor_tensor(ot[:], y3[m][:, sl], s3b[:],
                                            ALU.mult)
                    if m == 0:
                        nc.vector.tensor_copy(ot[0:1, :], cosh_c[:])
                    nc.sync.dma_start(C["out"].ap()[m * P:(m + 1) * P, sl],
                                      ot[:])

            deferred_tail = chunk_tail
        deferred_tail()
        t2p.release()
        y3p.release()

    body()
    for p in (outp, psn, psy, wp, accp, sqp, bcast, scalS, scalL, const):
        p.release()


# =====================================================================
# host-side prep + entry point
# =====================================================================

def _block_weight(w):
    """W [dout, din] f32 -> [mt*128, din] bf16 with row m*128+p holding, for
    each k-tile, lhsT tile (k,m) row p: out[m*128+p, k*128+j] = W.T[k*128+p,
    m*128+j].  One fully-contiguous [128, kt*128] DMA per m-tile."""
    dout, din = w.shape
    mt, kt = dout // P, din // P
    w = np.asarray(w, dtype=np.float32)
    blocked = (w.reshape(mt, P, kt, P)       # [m, j, k, p]
                .transpose(0, 3, 2, 1)       # [m, p, k, j]
                .reshape(mt * P, din))
    return np.ascontiguousarray(blocked.astype(ml_dtypes.bfloat16))


def _prep_bias(b, mt):
    """b [d] -> [128, mt] f32 with out[p, m] = b[m*128+p]."""
    return np.ascontiguousarray(
        np.asarray(b, dtype=np.float32).reshape(mt, P).T)


@functools.lru_cache(maxsize=2)
def _get_nc(fold=True):
    return build_nc_fold() if fold else build_nc_general()


def prep_in_maps_fold(x_hyp, W1, W2, W3):
    """logmap0 on the host: t0 = [0, d*xs/||xs||] feature-major bf16."""
    w1b = _block_weight(W1)
    w2b = _block_weight(W2)
    w3b = _block_weight(W3)
    x = np.asarray(x_hyp, dtype=np.float32)
    d_all = np.arccosh(np.maximum(x[:, 0], 1.0 + EPS))          # [TOK]
    xs_all = x[:, 1:]                                           # [TOK, 1023]
    ns_all = np.maximum(np.linalg.norm(xs_all, axis=1), EPS)
    s0_all = (d_all / ns_all).astype(np.float32)                # [TOK]
    in_maps = []
    for c in range(N_CORES):
        rows = slice(c * TOKPC, (c + 1) * TOKPC)
        t0 = np.zeros((D_IN, TOKPC), dtype=np.float32)
        t0[1:, :] = (xs_all[rows] * s0_all[rows, None]).T
        xt = t0.astype(ml_dtypes.bfloat16)
        in_maps.append(dict(xt=np.ascontiguousarray(xt),
                            w1=w1b, w2=w2b, w3=w3b))
    return in_maps


def prep_in_maps_general(x_hyp, W1, b1, W2, b2, W3, b3):
    w1b = _block_weight(W1)
    w2b = _block_weight(W2)
    w3b = _block_weight(W3)
    b1c = _prep_bias(b1, D_HID // P)
    b2c = _prep_bias(b2, D_HID // P)
    b3c = _prep_bias(b3, D_OUT // P)
    x = np.asarray(x_hyp, dtype=np.float32)
    in_maps = []
    for c in range(N_CORES):
        shard = x[c * TOKPC:(c + 1) * TOKPC, :]  # [tokpc, din]
        xt = shard.T.astype(ml_dtypes.bfloat16)
        xt[0, :] = 0  # zero time component (norm + GEMM both want it out)
        # x0 slot carries d = arccosh(clip(x0)) precomputed on host
        d = np.arccosh(np.maximum(shard[:, 0:1], 1.0 + EPS)).T
        in_maps.append(dict(xt=np.ascontiguousarray(xt),
                            x0=np.ascontiguousarray(d.astype(np.float32)),
                            w1=w1b, w2=w2b, w3=w3b,
                            b1=b1c, b2=b2c, b3=b3c))
    return in_maps


LAST_RESULTS = None


def kernel(x_hyp, W1, b1, W2, b2, W3, b3):
    global LAST_RESULTS
    fold = not (np.any(b1) or np.any(b2) or np.any(b3))
    nc = _get_nc(fold)
    if fold:
        in_maps = prep_in_maps_fold(x_hyp, W1, W2, W3)
    else:
        in_maps = prep_in_maps_general(x_hyp, W1, b1, W2, b2, W3, b3)
    res = bass_utils.run_bass_kernel_spmd(nc, in_maps,
                                          core_ids=list(range(N_CORES)))
    LAST_RESULTS = res
    parts = [np.asarray(res.results[c]["out"]).T for c in range(N_CORES)]
    return np.ascontiguousarray(np.concatenate(parts, axis=0),
                                dtype=np.float32)


# revision 17
# speedup vs baseline: 1.0141x; 1.0141x over previous
"""Trainium2 Bass kernel for a 3-layer Lorentz (hyperboloid) MLP.

Math: the reference chains lorentz_linear + inter-layer projx(expmap0(logmap0(.))).
Algebraically, expmap0 -> projx -> logmap0 round-trips cancel: the inter-layer op
on the tangent vector y is exactly "zero the time component, clamp the row norm
of y[1:] to 10".  So the network is:

  t0 = logmap0(x)                       (row scale d/||xs|| on xs, time comp 0)
  y1 = t0 @ W1.T + b1 ; t1 = clamp(y1)  (zero col 0, clamp row norm to 10)
  y2 = t1 @ W2.T + b2 ; t2 = clamp(y2)
  y3 = t2 @ W3.T + b3
  out = [cosh(nc), sinh(nc)/n * y3[1:]] with n=clip(||y3[1:]||,eps), nc=min(n,10)

With zero biases (the shipped case), per-token scales commute through the
GEMMs; logmap0 is applied on the host (t0 shipped pre-scaled in bf16) and the
inter-layer clamps are folded into a cumulative SQUARED per-token scale cs2
tracked on [1,tok] vectors, applied once at the very end.  The PE runs the
three GEMMs back to back with no inter-layer barrier.

All scalar-engine transcendentals (Ln, Exp, Identity, Square) live in the
single `natural_log_exp_and_others` table set, so there are no mid-kernel
ACT_TABLE_LOADs; rsqrt/sqrt are computed as exp(+-0.5*ln(x)) which is also
far more accurate than the Sqrt table (65536-ULP budget).

Layout: everything on-chip is FEATURE-major ([feat, token]); weights are
pre-transposed/blocked/bf16-cast on the host so each m-tile loads with one
fully contiguous DMA.  Row-wise (per-token) sum-of-squares are ones-vector
matmuls on the TensorEngine (partition-dim reduction), deferred one m-tile
into the next layer's PE stream so the PE never stalls on them.  GpSimd does
nothing but the two final partition_broadcasts (single ucode lib, loaded once
during the MM stream).

Sharding: pure data-parallel over tokens - 8192 tokens -> 8 cores x 1024.

With nonzero biases a general barrier path (scale applied between layers,
arccosh on device) is kept as fallback.
"""

import math
import os
import sys
import functools

import numpy as np
import ml_dtypes


def _import_concourse():
    try:
        import concourse  # noqa: F401
    except ImportError:
        for p in ("/opt/trn_rl_repo", "/root/.axon_site/_ro/trn_rl_repo"):
            if os.path.isdir(p) and p not in sys.path:
                sys.path.insert(0, p)
        import concourse  # noqa: F401


_import_concourse()

import concourse.bass as bass  # noqa: E402,F401
import concourse.bacc as bacc  # noqa: E402
import concourse.mybir as mybir  # noqa: E402
import concourse.tile as tile  # noqa: E402
from concourse import bass_utils  # noqa: E402

F32 = mybir.dt.float32
BF16 = mybir.dt.bfloat16
AF = mybir.ActivationFunctionType
ALU = mybir.AluOpType

P = 128
N_CORES = 8
EPS = 1e-7
EPS2 = 1e-14  # EPS**2, the ln(ssq + eps^2) guard
MAX_TAN_NORM = 10.0
LN10 = math.log(10.0)
LNHALF = math.log(0.5)

# Full-problem dims (hardcoded per spec)
TOK, D_IN, D_HID, D_OUT = 8192, 1024, 4096, 1024
TOKPC = TOK // N_CORES  # tokens per core


# =====================================================================
# Fold-mode program (zero biases - the shipped case)
# =====================================================================

def build_nc_fold(tokpc=TOKPC, din=D_IN, dhid=D_HID, dout=D_OUT, ch=512):
    assert tokpc % ch == 0
    nch = tokpc // ch
    kt1, mt1 = din // P, dhid // P
    kt2, mt2 = dhid // P, dhid // P
    kt3, mt3 = dhid // P, dout // P

    nc = bacc.Bacc("TRN2", target_bir_lowering=False, debug=False,
                   num_devices=N_CORES)

    xt_d = nc.dram_tensor("xt", [din, tokpc], BF16, kind="ExternalInput")
    w1_d = nc.dram_tensor("w1", [mt1 * P, din], BF16, kind="ExternalInput")
    w2_d = nc.dram_tensor("w2", [mt2 * P, dhid], BF16, kind="ExternalInput")
    w3_d = nc.dram_tensor("w3", [mt3 * P, dhid], BF16, kind="ExternalInput")
    out_d = nc.dram_tensor("out", [dout, tokpc], F32, kind="ExternalOutput")

    with tile.TileContext(nc) as tc:
        _build_fold_program(tc, nc, dict(
            tokpc=tokpc, ch=ch, nch=nch,
            kt1=kt1, mt1=mt1, kt2=kt2, mt2=mt2, kt3=kt3, mt3=mt3,
            xt=xt_d, w1=w1_d, w2=w2_d, w3=w3_d, out=out_d,
        ))
    nc.compile()
    return nc


def _build_fold_program(tc, nc, C):
    tokpc, ch, nch = C["tokpc"], C["ch"], C["nch"]

    const = tc.alloc_tile_pool(name="const", bufs=1)
    scalL = tc.alloc_tile_pool(name="scalL", bufs=3)   # [1, tokpc] f32
    scalS = tc.alloc_tile_pool(name="scalS", bufs=6)   # [1, ch] f32
    sqp = tc.alloc_tile_pool(name="sq", bufs=2)       # f32 (L3 only)
    sqbp = tc.alloc_tile_pool(name="sqb", bufs=2)     # bf16 (L1/L2)
    accp = tc.alloc_tile_pool(name="acc", bufs=2)     # f32 acc3 (L3)
    accbp = tc.alloc_tile_pool(name="accb", bufs=4)   # bf16 accs (L1/L2)
    wp = tc.alloc_tile_pool(name="wt", bufs=3)
    psy = tc.alloc_tile_pool(name="psy", bufs=6, space="PSUM")
    psn = tc.alloc_tile_pool(name="psn", bufs=2, space="PSUM")

    ones_f = const.tile([P, 1], F32, tag="ones_f", name="ones_f")
    nc.vector.memset(ones_f[:], 1.0)
    ones_k = const.tile([P, 1], BF16, tag="ones_k", name="ones_k")
    nc.vector.memset(ones_k[:], 1.0)
    c_eps2 = const.tile([P, 1], F32, tag="c_eps2", name="c_eps2")
    nc.vector.memset(c_eps2[:], EPS2)
    c_ln10 = const.tile([P, 1], F32, tag="c_ln10", name="c_ln10")
    nc.vector.memset(c_ln10[:], LN10)
    c_lnhalf = const.tile([P, 1], F32, tag="c_lnhalf", name="c_lnhalf")
    nc.vector.memset(c_lnhalf[:], LNHALF)

    # ---- head: first GEMM's gating DMAs go out first, split into
    # partition-halves for DMA-ring parallelism ----
    t0p = tc.alloc_tile_pool(name="t0", bufs=1, side="right")
    w1_pre = []
    t0 = []

    wm0 = wp.tile([P, C["kt1"] * P], BF16, tag="wtile", name="wm0")
    t = t0p.tile([P, tokpc], BF16, tag="t0_0", name="t0_0")
    H = P // 2
    nc.sync.dma_start(t[0:H, :], C["xt"].ap()[0:H, :])
    nc.sync.dma_start(wm0[0:H, :], C["w1"].ap()[0:H, :])
    nc.sync.dma_start(t[H:P, :], C["xt"].ap()[H:P, :])
    nc.sync.dma_start(wm0[H:P, :], C["w1"].ap()[H:P, :])
    w1_pre.append(wm0)
    t0.append(t)
    wm1 = wp.tile([P, C["kt1"] * P], BF16, tag="wtile", name="wm1")
    nc.sync.dma_start(wm1[:], C["w1"].ap()[P:2 * P, :])
    w1_pre.append(wm1)
    for k in range(1, C["kt1"]):
        t = t0p.tile([P, tokpc], BF16, tag=f"t0_{k}", name=f"t0_{k}")
        nc.sync.dma_start(t[:], C["xt"].ap()[k * P:(k + 1) * P, :])
        t0.append(t)

    # ---- HAM warmup: ~10 dummy matmuls during the DMA window get the PE
    # clock-gate to K=8/8 before the first real matmul arrives ----
    warm_src = const.tile([P, ch], BF16, tag="warm", name="warm")
    nc.vector.memset(warm_src[:], 1.0)
    ps_warm = psn.tile([1, ch], F32, tag="psn", name="ps_warm")
    for _ in range(12):
        nc.tensor.matmul(ps_warm[:], warm_src[:, 0:1], warm_src[:],
                         start=True, stop=True)

    def stile_s():
        return scalS.tile([1, ch], F32, tag="ss", name="ss")

    S = {"cs2": None}

    def gemm_layer(tin, w_d, kt, mt, out_pool, tag, mid_fn=None,
                   preloaded=()):
        """y[m] = sum_k w[k,m].T @ tin[k]; ACT evicts and squares straight
        from PSUM; squares accumulate on the DVE (f32) and a single fp32
        ones-matmul per chunk (deferred via finish()) does the final
        partition-reduce.  mid_fn is emitted after m==1's matmuls."""
        accs = [accbp.tile([P, ch], BF16, tag="accb", name=f"acc{_}")
                for _ in range(nch)]
        tout = []
        for m in range(mt):
            if m < len(preloaded):
                wm = preloaded[m]
            else:
                wm = wp.tile([P, kt * P], BF16, tag="wtile", name="wm")
                nc.sync.dma_start(wm[:], w_d.ap()[m * P:(m + 1) * P, :])
            pss = [psy.tile([P, ch], F32, tag="psy", name=f"psy{_}")
                   for _ in range(nch)]
            for k in range(kt):
                for c in range(nch):
                    nc.tensor.matmul(pss[c][:], wm[:, k * P:(k + 1) * P],
                                     tin[k][:, c * ch:(c + 1) * ch],
                                     start=(k == 0), stop=(k == kt - 1))
            if m == 1 and mid_fn is not None:
                mid_fn()
            ty = out_pool.tile([P, tokpc], BF16, tag=f"{tag}{m}",
                               name=f"{tag}{m}")
            for c in range(nch):
                if m == 0:
                    nc.scalar.activation(accs[c][:], pss[c][:], AF.Square)
                    nc.vector.memset(accs[c][0:1, :], 0.0)
                else:
                    sq = sqbp.tile([P, ch], BF16, tag="sqb", name="sq")
                    nc.scalar.activation(sq[:], pss[c][:], AF.Square)
                    nc.vector.tensor_tensor(accs[c][:], accs[c][:], sq[:],
                                            ALU.add)
                nc.scalar.activation(ty[:, c * ch:(c + 1) * ch], pss[c][:],
                                     AF.Identity)
            if m == 0:
                nc.vector.memset(ty[0:1, :], 0.0)
            tout.append(ty)

        def finish():
            ps_norm = [psn.tile([1, ch], F32, tag="psn", name=f"psn{_}")
                       for _ in range(nch)]
            for c in range(nch):
                nc.tensor.matmul(ps_norm[c][:], ones_k[:], accs[c][:],
                                 start=True, stop=True)
            return ps_norm
        return tout, finish

    def boundary(fin, first):
        """Inter-layer clamp folded into cs2 (squared cumulative scale).
        f = min(1, 10/sqrt(cs2_prev*ssq_raw)); cs2_new = cs2_prev * f^2.
        rsqrt via exp(-0.5*ln(.)) - Ln pass then Exp pass so the act-table
        set switches at most twice, not per chunk."""
        ps_norm = fin()
        cs2_new = scalL.tile([1, tokpc], F32, tag="cs2", name="cs2")
        Ls = []
        for c in range(nch):
            sl = slice(c * ch, (c + 1) * ch)
            if first:
                src = ps_norm[c][:]
            else:
                m2 = stile_s()
                nc.vector.tensor_tensor(m2[:], ps_norm[c][:],
                                        S["cs2"][:, sl], ALU.mult)
                src = m2[:]
            L = stile_s()
            nc.scalar.activation(L[:], src, AF.Ln, bias=c_eps2[0:1, :])
            Ls.append(L)
        for c in range(nch):
            sl = slice(c * ch, (c + 1) * ch)
            r10 = stile_s()
            nc.scalar.activation(r10[:], Ls[c][:], AF.Exp, scale=-0.5,
                                 bias=c_ln10[0:1, :])
            f = stile_s()
            nc.vector.tensor_scalar_min(f[:], r10[:], 1.0)
            if first:
                nc.vector.tensor_tensor(cs2_new[:, sl], f[:], f[:], ALU.mult)
            else:
                ff = stile_s()
                nc.vector.tensor_tensor(ff[:], f[:], f[:], ALU.mult)
                nc.vector.tensor_tensor(cs2_new[:, sl], S["cs2"][:, sl],
                                        ff[:], ALU.mult)
        S["cs2"] = cs2_new

    # ---------- Layers 1, 2 ----------
    t1p = tc.alloc_tile_pool(name="t1", bufs=1, side="left")
    t1, fin1 = gemm_layer(t0, C["w1"], C["kt1"], C["mt1"], t1p, "t1_",
                          preloaded=w1_pre)
    t0p.release()

    t2p = tc.alloc_tile_pool(name="t2", bufs=1, side="right")
    t2, fin2 = gemm_layer(t1, C["w2"], C["kt2"], C["mt2"], t2p, "t2_",
                          mid_fn=lambda: boundary(fin1, first=True))
    t1p.release()

    # ---------- Layer 3 + expmap0/projx, uneven chunks [512, 256, 256] so
    # the only exposed (last) tail is narrow; earlier tails overlap the next
    # chunk's matmuls ----------
    kt, mt = C["kt3"], C["mt3"]
    bcast = tc.alloc_tile_pool(name="bcast", bufs=2)
    outp = tc.alloc_tile_pool(name="outp", bufs=8)
    l3_chunks = []
    st = 0
    for w in [ch] * nch:
        l3_chunks.append((st, w))
        st += w
    assert st == tokpc
    y3p = tc.alloc_tile_pool(name="y3", bufs=1, side="left")
    y3 = [y3p.tile([P, tokpc], F32, tag=f"y3_{m}", name=f"y3_{m}")
          for m in range(mt)]
    deferred_tail = None
    for ci, (st, w) in enumerate(l3_chunks):
        sl = slice(st, st + w)
        acc3 = accp.tile([P, w], F32, tag="acc", name=f"acc3_{ci}")
        for m in range(mt):
            wm = wp.tile([P, kt * P], BF16, tag="wtile", name="wm")
            nc.sync.dma_start(wm[:], C["w3"].ap()[m * P:(m + 1) * P, :])
            ps = psy.tile([P, w], F32, tag="psy", name="psy3")
            for k in range(kt):
                nc.tensor.matmul(ps[:], wm[:, k * P:(k + 1) * P],
                                 t2[k][:, sl],
                                 start=(k == 0), stop=(k == kt - 1))
            if m == 1:
                if ci == 0:
                    boundary(fin2, first=False)
                    # cs = sqrt(cs2) via exp(0.5*ln(.)), off the critical
                    # path - the final tails then need only ONE Ln each
                    cs = scalL.tile([1, tokpc], F32, tag="cs", name="cs")
                    for cc in range(nch):
                        ssl = slice(cc * ch, (cc + 1) * ch)
                        Lc = stile_s()
                        nc.scalar.activation(Lc[:], S["cs2"][:, ssl], AF.Ln)
                        nc.scalar.activation(cs[:, ssl], Lc[:], AF.Exp,
                                             scale=0.5)
                    S["cs"] = cs
                if deferred_tail is not None:
                    deferred_tail()
                    deferred_tail = None
            if m == 0:
                nc.scalar.activation(acc3[:], ps[:], AF.Square)
                nc.vector.memset(acc3[0:1, :], 0.0)
            else:
                sq = sqp.tile([P, w], F32, tag="sq", name="sq")
                nc.scalar.activation(sq[:], ps[:], AF.Square)
                nc.vector.tensor_tensor(acc3[:], acc3[:], sq[:], ALU.add)
            nc.scalar.activation(y3[m][:, sl], ps[:], AF.Identity)

        def chunk_tail(sl=sl, w=w, acc3=acc3, ci=ci):
            # true norm n = sqrt(cs2*ssq_raw); nc = min(n, 10)
            # out[0] = cosh(nc) = 0.5 e^nc + 0.5 e^-nc
            # out[1:] = y3_raw * sinh(nc) / sqrt(ssq_raw)
            # Ln calls grouped before Exp calls: 2 act-table switches total.
            def stile_w():
                return scalS.tile([1, w], F32, tag="ss", name="ss")
            ps_norm = psn.tile([1, w], F32, tag="psn", name=f"psn3_{ci}")
            nc.tensor.matmul(ps_norm[:], ones_f[:], acc3[:],
                             start=True, stop=True)
            m2 = stile_w()
            nc.vector.tensor_tensor(m2[:], ps_norm[:], S["cs2"][:, sl],
                                    ALU.mult)
            Lm = stile_w()
            nc.scalar.activation(Lm[:], m2[:], AF.Ln, bias=c_eps2[0:1, :])
            inv_n = stile_w()
            nc.scalar.activation(inv_n[:], Lm[:], AF.Exp, scale=-0.5)
            n_ = stile_w()
            nc.vector.tensor_tensor(n_[:], m2[:], inv_n[:], ALU.mult)
            ncl = stile_w()
            nc.vector.tensor_scalar_min(ncl[:], n_[:], MAX_TAN_NORM)
            ic = stile_w()
            nc.vector.tensor_tensor(ic[:], inv_n[:], S["cs"][:, sl],
                                    ALU.mult)
            e_ = stile_w()
            nc.scalar.activation(e_[:], ncl[:], AF.Exp, scale=1.0,
                                 bias=c_lnhalf[0:1, :])
            en = stile_w()
            nc.scalar.activation(en[:], ncl[:], AF.Exp, scale=-1.0,
                                 bias=c_lnhalf[0:1, :])
            sh = stile_w()
            nc.vector.tensor_tensor(sh[:], e_[:], en[:], ALU.subtract)
            s3 = stile_w()
            nc.vector.tensor_tensor(s3[:], sh[:], ic[:], ALU.mult)
            cosh_c = stile_w()
            nc.vector.tensor_tensor(cosh_c[:], e_[:], en[:], ALU.add)
            s3b = bcast.tile([P, w], F32, tag="s3b", name="s3b")
            nc.gpsimd.partition_broadcast(s3b[:], s3[:])
            for m in range(mt):
                ot = outp.tile([P, w], F32, tag="ot", name="ot")
                nc.vector.tensor_tensor(ot[:], y3[m][:, sl], s3b[:],
                                        ALU.mult)
                if m == 0:
                    nc.vector.tensor_copy(ot[0:1, :], cosh_c[:])
                nc.gpsimd.dma_start(C["out"].ap()[m * P:(m + 1) * P, sl],
                                    ot[:])

        deferred_tail = chunk_tail
    deferred_tail()
    y3p.release()
    outp.release()
    bcast.release()
    t2p.release()

    for p in (psn, psy, wp, accbp, accp, sqbp, sqp, scalS, scalL, const):
        p.release()


# =====================================================================
# General (nonzero-bias) fallback program - barrier between layers
# =====================================================================

def build_nc_general(tokpc=TOKPC, din=D_IN, dhid=D_HID, dout=D_OUT, ch=512):
    assert tokpc % ch == 0
    nch = tokpc // ch
    kt1, mt1 = din // P, dhid // P
    kt2, mt2 = dhid // P, dhid // P
    kt3, mt3 = dhid // P, dout // P

    nc = bacc.Bacc("TRN2", target_bir_lowering=False, debug=False,
                   num_devices=N_CORES)

    xt_d = nc.dram_tensor("xt", [din, tokpc], BF16, kind="ExternalInput")
    x0_d = nc.dram_tensor("x0", [1, tokpc], F32, kind="ExternalInput")
    w1_d = nc.dram_tensor("w1", [mt1 * P, din], BF16, kind="ExternalInput")
    w2_d = nc.dram_tensor("w2", [mt2 * P, dhid], BF16, kind="ExternalInput")
    w3_d = nc.dram_tensor("w3", [mt3 * P, dhid], BF16, kind="ExternalInput")
    b1_d = nc.dram_tensor("b1", [P, mt1], F32, kind="ExternalInput")
    b2_d = nc.dram_tensor("b2", [P, mt2], F32, kind="ExternalInput")
    b3_d = nc.dram_tensor("b3", [P, mt3], F32, kind="ExternalInput")
    out_d = nc.dram_tensor("out", [dout, tokpc], F32, kind="ExternalOutput")

    with tile.TileContext(nc) as tc:
        _build_general_program(tc, nc, dict(
            tokpc=tokpc, din=din, dhid=dhid, dout=dout, ch=ch, nch=nch,
            kt1=kt1, mt1=mt1, kt2=kt2, mt2=mt2, kt3=kt3, mt3=mt3,
            xt=xt_d, x0=x0_d, w1=w1_d, w2=w2_d, w3=w3_d,
            b1=b1_d, b2=b2_d, b3=b3_d, out=out_d,
        ))
    nc.compile()
    return nc


def _build_general_program(tc, nc, C):
    tokpc, ch, nch = C["tokpc"], C["ch"], C["nch"]

    const = tc.alloc_tile_pool(name="const", bufs=1)
    scalL = tc.alloc_tile_pool(name="scalL", bufs=5)
    scalS = tc.alloc_tile_pool(name="scalS", bufs=6)
    bcast = tc.alloc_tile_pool(name="bcast", bufs=2)
    sqp = tc.alloc_tile_pool(name="sq", bufs=2)
    accp = tc.alloc_tile_pool(name="acc", bufs=4)
    wp = tc.alloc_tile_pool(name="wt", bufs=3)
    psy = tc.alloc_tile_pool(name="psy", bufs=4, space="PSUM")
    psn = tc.alloc_tile_pool(name="psn", bufs=4, space="PSUM")
    outp = tc.alloc_tile_pool(name="outp", bufs=4)

    bias1 = const.tile([P, C["mt1"]], F32, tag="bias1")
    nc.sync.dma_start(bias1[:], C["b1"].ap())
    bias2 = const.tile([P, C["mt2"]], F32, tag="bias2")
    nc.sync.dma_start(bias2[:], C["b2"].ap())
    bias3 = const.tile([P, C["mt3"]], F32, tag="bias3")
    nc.sync.dma_start(bias3[:], C["b3"].ap())
    ones_f = const.tile([P, 1], F32, tag="ones_f", name="ones_f")
    nc.vector.memset(ones_f[:], 1.0)

    def stile_l():
        return scalL.tile([1, tokpc], F32, tag="sl", name="sl")

    def stile_s():
        return scalS.tile([1, ch], F32, tag="ss", name="ss")

    def norm_accum_tiles():
        return [psn.tile([1, ch], F32, tag="psn", name=f"psn{_}")
                for _ in range(nch)]

    def bcast_full(s_full):
        sb = bcast.tile([P, tokpc], F32, tag="sb", name="sb")
        nc.gpsimd.partition_broadcast(sb[:], s_full[:])
        return sb

    def gemm_layer(tin, w_d, bias_t, kt, mt, out_pool, out_dtype, tag):
        accs = [accp.tile([P, ch], F32, tag="acc", name=f"acc{_}")
                for _ in range(nch)]
        tout = []
        for m in range(mt):
            wm = wp.tile([P, kt * P], BF16, tag="wtile", name="wm")
            nc.sync.dma_start(wm[:], w_d.ap()[m * P:(m + 1) * P, :])
            pss = [psy.tile([P, ch], F32, tag="psy", name=f"psy{_}")
                   for _ in range(nch)]
            for k in range(kt):
                for c in range(nch):
                    nc.tensor.matmul(pss[c][:], wm[:, k * P:(k + 1) * P],
                                     tin[k][:, c * ch:(c + 1) * ch],
                                     start=(k == 0), stop=(k == kt - 1))
            ty = out_pool.tile([P, tokpc], out_dtype, tag=f"{tag}{m}",
                               name=f"{tag}{m}")
            for c in range(nch):
                nc.scalar.activation(ty[:, c * ch:(c + 1) * ch], pss[c][:],
                                     AF.Identity, bias=bias_t[:, m:m + 1],
                                     scale=1.0)
                if m == 0:
                    nc.scalar.activation(accs[c][:], pss[c][:], AF.Square,
                                         bias=bias_t[:, m:m + 1], scale=1.0)
                    nc.vector.memset(accs[c][0:1, :], 0.0)
                else:
                    sq = sqp.tile([P, ch], F32, tag="sq", name="sq")
                    nc.scalar.activation(sq[:], pss[c][:], AF.Square,
                                         bias=bias_t[:, m:m + 1], scale=1.0)
                    nc.vector.tensor_tensor(accs[c][:], accs[c][:], sq[:],
                                            ALU.add)
            if m == 0:
                nc.vector.memset(ty[0:1, :], 0.0)
            tout.append(ty)

        def finish():
            ps_norm = norm_accum_tiles()
            for c in range(nch):
                nc.tensor.matmul(ps_norm[c][:], ones_f[:], accs[c][:],
                                 start=True, stop=True)
            return ps_norm
        return tout, finish

    def clamp_scale(ps_norm):
        """s = min(max(sqrt(ssq),eps),10)/max(sqrt(ssq),eps) via ln/exp."""
        s = stile_l()
        for c in range(nch):
            L = stile_s()
            nc.scalar.activation(L[:], ps_norm[c][:], AF.Ln, bias=EPS2)
            r10 = stile_s()
            nc.scalar.activation(r10[:], L[:], AF.Exp, scale=-0.5, bias=LN10)
            f = stile_s()
            nc.vector.tensor_scalar_min(f[:], r10[:], 1.0)
            nc.vector.tensor_copy(s[:, c * ch:(c + 1) * ch], f[:])
        return s

    def apply_scale(tiles, sb):
        for t in tiles:
            nc.vector.tensor_tensor(t[:], t[:], sb[:], ALU.mult)

    def body():
        # Phase 0: load bf16 xs (= raw t0), input norm, s0
        t0p = tc.alloc_tile_pool(name="t0", bufs=1, side="right")
        t0 = []
        for k in range(C["kt1"]):
            t = t0p.tile([P, tokpc], BF16, tag=f"t0_{k}", name=f"t0_{k}")
            nc.sync.dma_start(t[:], C["xt"].ap()[k * P:(k + 1) * P, :])
            t0.append(t)
        x0t = stile_l()
        nc.sync.dma_start(x0t[:], C["x0"].ap())

        acc0 = [accp.tile([P, ch], F32, tag="acc", name=f"acc0_{_}")
                for _ in range(nch)]
        for k in range(C["kt1"]):
            for c in range(nch):
                if k == 0:
                    nc.scalar.activation(acc0[c][:],
                                         t0[k][:, c * ch:(c + 1) * ch],
                                         AF.Square)
                else:
                    sq = sqp.tile([P, ch], F32, tag="sq", name="sq")
                    nc.scalar.activation(sq[:], t0[k][:, c * ch:(c + 1) * ch],
                                         AF.Square)
                    nc.vector.tensor_tensor(acc0[c][:], acc0[c][:], sq[:],
                                            ALU.add)

        # norm-MM for the input + s0 = arccosh(x0) / ||xs|| chain
        # (x0 input here is pre-clipped arccosh distance d, computed on host)
        ps_n0 = norm_accum_tiles()
        for c in range(nch):
            nc.tensor.matmul(ps_n0[c][:], ones_f[:], acc0[c][:],
                             start=True, stop=True)
        s0 = stile_l()
        for c in range(nch):
            L = stile_s()
            nc.scalar.activation(L[:], ps_n0[c][:], AF.Ln, bias=EPS2)
            r = stile_s()
            nc.scalar.activation(r[:], L[:], AF.Exp, scale=-0.5)
            nc.vector.tensor_tensor(s0[:, c * ch:(c + 1) * ch],
                                    x0t[:, c * ch:(c + 1) * ch], r[:],
                                    ALU.mult)

        s0b = bcast_full(s0)
        for k in range(C["kt1"]):
            nc.vector.tensor_tensor(t0[k][:], t0[k][:], s0b[:], ALU.mult)

        # Layers 1, 2 with barrier scale application
        t1p = tc.alloc_tile_pool(name="t1", bufs=1, side="left")
        t1, fin1 = gemm_layer(t0, C["w1"], bias1, C["kt1"], C["mt1"],
                              t1p, BF16, "t1_")
        apply_scale(t1, bcast_full(clamp_scale(fin1())))
        t0p.release()

        t2p = tc.alloc_tile_pool(name="t2", bufs=1, side="right")
        t2, fin2 = gemm_layer(t1, C["w2"], bias2, C["kt2"], C["mt2"],
                              t2p, BF16, "t2_")
        apply_scale(t2, bcast_full(clamp_scale(fin2())))
        t1p.release()

        # Layer 3 + expmap0/projx
        kt, mt = C["kt3"], C["mt3"]
        y3p = tc.alloc_tile_pool(name="y3", bufs=1, side="left")
        y3 = [y3p.tile([P, tokpc], F32, tag=f"y3_{m}", name=f"y3_{m}")
              for m in range(mt)]
        deferred_tail = None
        for c in range(nch):
            sl = slice(c * ch, (c + 1) * ch)
            acc3 = accp.tile([P, ch], F32, tag="acc", name=f"acc3_{c}")
            for m in range(mt):
                wm = wp.tile([P, kt * P], BF16, tag="wtile", name="wm")
                nc.sync.dma_start(wm[:], C["w3"].ap()[m * P:(m + 1) * P, :])
                ps = psy.tile([P, ch], F32, tag="psy", name="psy3")
                for k in range(kt):
                    nc.tensor.matmul(ps[:], wm[:, k * P:(k + 1) * P],
                                     t2[k][:, sl],
                                     start=(k == 0), stop=(k == kt - 1))
                if m == 1 and deferred_tail is not None:
                    deferred_tail()
                    deferred_tail = None
                nc.scalar.activation(y3[m][:, sl], ps[:], AF.Identity,
                                     bias=bias3[:, m:m + 1], scale=1.0)
                if m == 0:
                    nc.scalar.activation(acc3[:], ps[:], AF.Square,
                                         bias=bias3[:, m:m + 1], scale=1.0)
                    nc.vector.memset(acc3[0:1, :], 0.0)
                else:
                    sq = sqp.tile([P, ch], F32, tag="sq", name="sq")
                    nc.scalar.activation(sq[:], ps[:], AF.Square,
                                         bias=bias3[:, m:m + 1], scale=1.0)
                    nc.vector.tensor_tensor(acc3[:], acc3[:], sq[:], ALU.add)

            def chunk_tail(sl=sl, acc3=acc3, c=c):
                ps_norm = psn.tile([1, ch], F32, tag="psn", name=f"psn3_{c}")
                nc.tensor.matmul(ps_norm[:], ones_f[:], acc3[:],
                                 start=True, stop=True)
                Lr = stile_s()
                nc.scalar.activation(Lr[:], ps_norm[:], AF.Ln, bias=EPS2)
                rr = stile_s()
                nc.scalar.activation(rr[:], Lr[:], AF.Exp, scale=-0.5)
                n_ = stile_s()
                nc.scalar.activation(n_[:], Lr[:], AF.Exp, scale=0.5)
                ncl = stile_s()
                nc.vector.tensor_scalar_min(ncl[:], n_[:], MAX_TAN_NORM)
                e_ = stile_s()
                nc.scalar.activation(e_[:], ncl[:], AF.Exp, scale=1.0,
                                     bias=LNHALF)
                en = stile_s()
                nc.scalar.activation(en[:], ncl[:], AF.Exp, scale=-1.0,
                                     bias=LNHALF)
                cosh_c = stile_s()
                nc.vector.tensor_tensor(cosh_c[:], e_[:], en[:], ALU.add)
                sh = stile_s()
                nc.vector.tensor_tensor(sh[:], e_[:], en[:], ALU.subtract)
                s3 = stile_s()
                nc.vector.tensor_tensor(s3[:], sh[:], rr[:], ALU.mult)
                s3b = bcast.tile([P, ch], F32, tag="s3b", name="s3b")
                nc.gpsimd.partition_broadcast(s3b[:], s3[:])
                for m in range(mt):
                    ot = outp.tile([P, ch], F32, tag="ot", name="ot")
                    nc.vector.tensor_tensor(ot[:], y3[m][:, sl], s3b[:],
                                            ALU.mult)
                    if m == 0:
                        nc.vector.tensor_copy(ot[0:1, :], cosh_c[:])
                    nc.sync.dma_start(C["out"].ap()[m * P:(m + 1) * P, sl],
                                      ot[:])

            deferred_tail = chunk_tail
        deferred_tail()
        t2p.release()
        y3p.release()

    body()
    for p in (outp, psn, psy, wp, accp, sqp, bcast, scalS, scalL, const):
        p.release()


# =====================================================================
# host-side prep + entry point
# =====================================================================

def _block_weight(w):
    """W [dout, din] f32 -> [mt*128, din] bf16 with row m*128+p holding, for
    each k-tile, lhsT tile (k,m) row p: out[m*128+p, k*128+j] = W.T[k*128+p,
    m*128+j].  One fully-contiguous [128, kt*128] DMA per m-tile."""
    dout, din = w.shape
    mt, kt = dout // P, din // P
    w = np.asarray(w, dtype=np.float32)
    blocked = (w.reshape(mt, P, kt, P)       # [m, j, k, p]
                .transpose(0, 3, 2, 1)       # [m, p, k, j]
                .reshape(mt * P, din))
    return np.ascontiguousarray(blocked.astype(ml_dtypes.bfloat16))


def _prep_bias(b, mt):
    """b [d] -> [128, mt] f32 with out[p, m] = b[m*128+p]."""
    return np.ascontiguousarray(
        np.asarray(b, dtype=np.float32).reshape(mt, P).T)


@functools.lru_cache(maxsize=2)
def _get_nc(fold=True):
    return build_nc_fold() if fold else build_nc_general()


def prep_in_maps_fold(x_hyp, W1, W2, W3):
    """logmap0 on the host: t0 = [0, d*xs/||xs||] feature-major bf16."""
    w1b = _block_weight(W1)
    w2b = _block_weight(W2)
    w3b = _block_weight(W3)
    x = np.asarray(x_hyp, dtype=np.float32)
    d_all = np.arccosh(np.maximum(x[:, 0], 1.0 + EPS))          # [TOK]
    xs_all = x[:, 1:]                                           # [TOK, 1023]
    ns_all = np.maximum(np.linalg.norm(xs_all, axis=1), EPS)
    s0_all = (d_all / ns_all).astype(np.float32)                # [TOK]
    in_maps = []
    for c in range(N_CORES):
        rows = slice(c * TOKPC, (c + 1) * TOKPC)
        t0 = np.zeros((D_IN, TOKPC), dtype=np.float32)
        t0[1:, :] = (xs_all[rows] * s0_all[rows, None]).T
        xt = t0.astype(ml_dtypes.bfloat16)
        in_maps.append(dict(xt=np.ascontiguousarray(xt),
                            w1=w1b, w2=w2b, w3=w3b))
    return in_maps


def prep_in_maps_general(x_hyp, W1, b1, W2, b2, W3, b3):
    w1b = _block_weight(W1)
    w2b = _block_weight(W2)
    w3b = _block_weight(W3)
    b1c = _prep_bias(b1, D_HID // P)
    b2c = _prep_bias(b2, D_HID // P)
    b3c = _prep_bias(b3, D_OUT // P)
    x = np.asarray(x_hyp, dtype=np.float32)
    in_maps = []
    for c in range(N_CORES):
        shard = x[c * TOKPC:(c + 1) * TOKPC, :]  # [tokpc, din]
        xt = shard.T.astype(ml_dtypes.bfloat16)
        xt[0, :] = 0  # zero time component (norm + GEMM both want it out)
        # x0 slot carries d = arccosh(clip(x0)) precomputed on host
        d = np.arccosh(np.maximum(shard[:, 0:1], 1.0 + EPS)).T
        in_maps.append(dict(xt=np.ascontiguousarray(xt),
                            x0=np.ascontiguousarray(d.astype(np.float32)),
                            w1=w1b, w2=w2b, w3=w3b,
                            b1=b1c, b2=b2c, b3=b3c))
    return in_maps


LAST_RESULTS = None


def kernel(x_hyp, W1, b1, W2, b2, W3, b3):
    global LAST_RESULTS
    fold = not (np.any(b1) or np.any(b2) or np.any(b3))
    nc = _get_nc(fold)
    if fold:
        in_maps = prep_in_maps_fold(x_hyp, W1, W2, W3)
    else:
        in_maps = prep_in_maps_general(x_hyp, W1, b1, W2, b2, W3, b3)
    res = bass_utils.run_bass_kernel_spmd(nc, in_maps,
                                          core_ids=list(range(N_CORES)))
    LAST_RESULTS = res
    parts = [np.asarray(res.results[c]["out"]).T for c in range(N_CORES)]
    return np.ascontiguousarray(np.concatenate(parts, axis=0),
                                dtype=np.float32)


# revision 25
# speedup vs baseline: 1.0150x; 1.0008x over previous
"""Trainium2 Bass kernel for a 3-layer Lorentz (hyperboloid) MLP.

Math: the reference chains lorentz_linear + inter-layer projx(expmap0(logmap0(.))).
Algebraically, expmap0 -> projx -> logmap0 round-trips cancel: the inter-layer op
on the tangent vector y is exactly "zero the time component, clamp the row norm
of y[1:] to 10".  So the network is:

  t0 = logmap0(x)                       (row scale d/||xs|| on xs, time comp 0)
  y1 = t0 @ W1.T + b1 ; t1 = clamp(y1)  (zero col 0, clamp row norm to 10)
  y2 = t1 @ W2.T + b2 ; t2 = clamp(y2)
  y3 = t2 @ W3.T + b3
  out = [cosh(nc), sinh(nc)/n * y3[1:]] with n=clip(||y3[1:]||,eps), nc=min(n,10)

With zero biases (the shipped case), per-token scales commute through the
GEMMs; logmap0 is applied on the host (t0 shipped pre-scaled in bf16) and the
inter-layer clamps are folded into a cumulative SQUARED per-token scale cs2
tracked on [1,tok] vectors, applied once at the very end.  The PE runs the
three GEMMs back to back with no inter-layer barrier.

All scalar-engine transcendentals (Ln, Exp, Identity, Square) live in the
single `natural_log_exp_and_others` table set, so there are no mid-kernel
ACT_TABLE_LOADs; rsqrt/sqrt are computed as exp(+-0.5*ln(x)) which is also
far more accurate than the Sqrt table (65536-ULP budget).

Layout: everything on-chip is FEATURE-major ([feat, token]); weights are
pre-transposed/blocked/bf16-cast on the host so each m-tile loads with one
fully contiguous DMA.  Row-wise (per-token) sum-of-squares are ones-vector
matmuls on the TensorEngine (partition-dim reduction), deferred one m-tile
into the next layer's PE stream so the PE never stalls on them.  GpSimd does
nothing but the two final partition_broadcasts (single ucode lib, loaded once
during the MM stream).

Sharding: pure data-parallel over tokens - 8192 tokens -> 8 cores x 1024.

With nonzero biases a general barrier path (scale applied between layers,
arccosh on device) is kept as fallback.
"""

import math
import os
import sys
import functools

import numpy as np
import ml_dtypes


def _import_concourse():
    try:
        import concourse  # noqa: F401
    except ImportError:
        for p in ("/opt/trn_rl_repo", "/root/.axon_site/_ro/trn_rl_repo"):
            if os.path.isdir(p) and p not in sys.path:
                sys.path.insert(0, p)
        import concourse  # noqa: F401


_import_concourse()

import concourse.bass as bass  # noqa: E402,F401
import concourse.bacc as bacc  # noqa: E402
import concourse.mybir as mybir  # noqa: E402
import concourse.tile as tile  # noqa: E402
from concourse import bass_utils  # noqa: E402

F32 = mybir.dt.float32
BF16 = mybir.dt.bfloat16
AF = mybir.ActivationFunctionType
ALU = mybir.AluOpType

P = 128
N_CORES = 8
EPS = 1e-7
EPS2 = 1e-14  # EPS**2, the ln(ssq + eps^2) guard
MAX_TAN_NORM = 10.0
LN10 = math.log(10.0)
LNHALF = math.log(0.5)

# Full-problem dims (hardcoded per spec)
TOK, D_IN, D_HID, D_OUT = 8192, 1024, 4096, 1024
TOKPC = TOK // N_CORES  # tokens per core


# =====================================================================
# Fold-mode program (zero biases - the shipped case)
# =====================================================================

def build_nc_fold(tokpc=TOKPC, din=D_IN, dhid=D_HID, dout=D_OUT, ch=512):
    assert tokpc % ch == 0
    nch = tokpc // ch
    kt1, mt1 = din // P, dhid // P
    kt2, mt2 = dhid // P, dhid // P
    kt3, mt3 = dhid // P, dout // P

    nc = bacc.Bacc("TRN2", target_bir_lowering=False, debug=False,
                   num_devices=N_CORES)

    xt_d = nc.dram_tensor("xt", [din, tokpc], BF16, kind="ExternalInput")
    w1_d = nc.dram_tensor("w1", [mt1 * P, din], BF16, kind="ExternalInput")
    w2_d = nc.dram_tensor("w2", [mt2 * P, dhid], BF16, kind="ExternalInput")
    w3_d = nc.dram_tensor("w3", [mt3 * P, dhid], BF16, kind="ExternalInput")
    out_d = nc.dram_tensor("out", [dout, tokpc], F32, kind="ExternalOutput")

    with tile.TileContext(nc) as tc:
        _build_fold_program(tc, nc, dict(
            tokpc=tokpc, ch=ch, nch=nch,
            kt1=kt1, mt1=mt1, kt2=kt2, mt2=mt2, kt3=kt3, mt3=mt3,
            xt=xt_d, w1=w1_d, w2=w2_d, w3=w3_d, out=out_d,
        ))
    nc.compile()
    return nc


def _build_fold_program(tc, nc, C):
    tokpc, ch, nch = C["tokpc"], C["ch"], C["nch"]

    const = tc.alloc_tile_pool(name="const", bufs=1)
    scalL = tc.alloc_tile_pool(name="scalL", bufs=2)   # [1, tokpc] f32
    scalS = tc.alloc_tile_pool(name="scalS", bufs=6)   # [1, ch] f32
    sqp = tc.alloc_tile_pool(name="sq", bufs=2)       # f32 (L3 only)
    sqbp = tc.alloc_tile_pool(name="sqb", bufs=2)     # bf16 (L1/L2)
    accp = tc.alloc_tile_pool(name="acc", bufs=3)     # f32 acc3 (L3)
    accbp = tc.alloc_tile_pool(name="accb", bufs=4)   # bf16 accs (L1/L2)
    wp = tc.alloc_tile_pool(name="wt", bufs=3)
    psy = tc.alloc_tile_pool(name="psy", bufs=6, space="PSUM")
    psn = tc.alloc_tile_pool(name="psn", bufs=2, space="PSUM")

    ones_f = const.tile([P, 1], F32, tag="ones_f", name="ones_f")
    nc.vector.memset(ones_f[:], 1.0)
    ones_k = const.tile([P, 1], BF16, tag="ones_k", name="ones_k")
    nc.vector.memset(ones_k[:], 1.0)
    c_eps2 = const.tile([P, 1], F32, tag="c_eps2", name="c_eps2")
    nc.vector.memset(c_eps2[:], EPS2)
    c_ln10 = const.tile([P, 1], F32, tag="c_ln10", name="c_ln10")
    nc.vector.memset(c_ln10[:], LN10)
    c_lnhalf = const.tile([P, 1], F32, tag="c_lnhalf", name="c_lnhalf")
    nc.vector.memset(c_lnhalf[:], LNHALF)

    # ---- head: first GEMM's gating DMAs go out first, split into
    # partition-halves for DMA-ring parallelism ----
    t0p = tc.alloc_tile_pool(name="t0", bufs=1, side="right")
    w1_pre = []
    t0 = []

    wm0 = wp.tile([P, C["kt1"] * P], BF16, tag="wtile", name="wm0")
    t = t0p.tile([P, tokpc], BF16, tag="t0_0", name="t0_0")
    H = P // 2
    nc.sync.dma_start(t[0:H, :], C["xt"].ap()[0:H, :])
    nc.sync.dma_start(wm0[0:H, :], C["w1"].ap()[0:H, :])
    nc.sync.dma_start(t[H:P, :], C["xt"].ap()[H:P, :])
    nc.sync.dma_start(wm0[H:P, :], C["w1"].ap()[H:P, :])
    w1_pre.append(wm0)
    t0.append(t)
    wm1 = wp.tile([P, C["kt1"] * P], BF16, tag="wtile", name="wm1")
    nc.sync.dma_start(wm1[:], C["w1"].ap()[P:2 * P, :])
    w1_pre.append(wm1)
    for k in range(1, C["kt1"]):
        t = t0p.tile([P, tokpc], BF16, tag=f"t0_{k}", name=f"t0_{k}")
        nc.sync.dma_start(t[:], C["xt"].ap()[k * P:(k + 1) * P, :])
        t0.append(t)

    # ---- HAM warmup: ~10 dummy matmuls during the DMA window get the PE
    # clock-gate to K=8/8 before the first real matmul arrives ----
    warm_src = const.tile([P, ch], BF16, tag="warm", name="warm")
    nc.vector.memset(warm_src[:], 1.0)
    ps_warm = psn.tile([1, ch], F32, tag="psn", name="ps_warm")
    for _ in range(12):
        nc.tensor.matmul(ps_warm[:], warm_src[:, 0:1], warm_src[:],
                         start=True, stop=True)

    def stile_s():
        return scalS.tile([1, ch], F32, tag="ss", name="ss")

    S = {"cs2": None}

    def gemm_layer(tin, w_d, kt, mt, out_pool, tag, mid_fn=None,
                   preloaded=()):
        """y[m] = sum_k w[k,m].T @ tin[k]; ACT evicts and squares straight
        from PSUM; squares accumulate on the DVE (f32) and a single fp32
        ones-matmul per chunk (deferred via finish()) does the final
        partition-reduce.  mid_fn is emitted after m==1's matmuls."""
        accs = [accbp.tile([P, ch], BF16, tag="accb", name=f"acc{_}")
                for _ in range(nch)]
        tout = []
        for m in range(mt):
            if m < len(preloaded):
                wm = preloaded[m]
            else:
                wm = wp.tile([P, kt * P], BF16, tag="wtile", name="wm")
                nc.sync.dma_start(wm[:], w_d.ap()[m * P:(m + 1) * P, :])
            pss = [psy.tile([P, ch], F32, tag="psy", name=f"psy{_}")
                   for _ in range(nch)]
            for k in range(kt):
                for c in range(nch):
                    nc.tensor.matmul(pss[c][:], wm[:, k * P:(k + 1) * P],
                                     tin[k][:, c * ch:(c + 1) * ch],
                                     start=(k == 0), stop=(k == kt - 1))
            if m == 1 and mid_fn is not None:
                mid_fn()
            ty = out_pool.tile([P, tokpc], BF16, tag=f"{tag}{m}",
                               name=f"{tag}{m}")
            for c in range(nch):
                if m == 0:
                    nc.scalar.activation(accs[c][:], pss[c][:], AF.Square)
                    nc.vector.memset(accs[c][0:1, :], 0.0)
                else:
                    sq = sqbp.tile([P, ch], BF16, tag="sqb", name="sq")
                    nc.scalar.activation(sq[:], pss[c][:], AF.Square)
                    nc.vector.tensor_tensor(accs[c][:], accs[c][:], sq[:],
                                            ALU.add)
                nc.scalar.activation(ty[:, c * ch:(c + 1) * ch], pss[c][:],
                                     AF.Identity)
            if m == 0:
                nc.vector.memset(ty[0:1, :], 0.0)
            tout.append(ty)

        def finish():
            ps_norm = [psn.tile([1, ch], F32, tag="psn", name=f"psn{_}")
                       for _ in range(nch)]
            for c in range(nch):
                nc.tensor.matmul(ps_norm[c][:], ones_k[:], accs[c][:],
                                 start=True, stop=True)
            return ps_norm
        return tout, finish

    def boundary(fin, first):
        """Inter-layer clamp folded into cs2 (squared cumulative scale).
        f = min(1, 10/sqrt(cs2_prev*ssq_raw)); cs2_new = cs2_prev * f^2.
        rsqrt via exp(-0.5*ln(.)) - Ln pass then Exp pass so the act-table
        set switches at most twice, not per chunk."""
        ps_norm = fin()
        cs2_new = scalL.tile([1, tokpc], F32, tag="cs2", name="cs2")
        Ls = []
        for c in range(nch):
            sl = slice(c * ch, (c + 1) * ch)
            if first:
                src = ps_norm[c][:]
            else:
                m2 = stile_s()
                nc.vector.tensor_tensor(m2[:], ps_norm[c][:],
                                        S["cs2"][:, sl], ALU.mult)
                src = m2[:]
            L = stile_s()
            nc.scalar.activation(L[:], src, AF.Ln, bias=c_eps2[0:1, :])
            Ls.append(L)
        for c in range(nch):
            sl = slice(c * ch, (c + 1) * ch)
            r10 = stile_s()
            nc.scalar.activation(r10[:], Ls[c][:], AF.Exp, scale=-0.5,
                                 bias=c_ln10[0:1, :])
            f = stile_s()
            nc.vector.tensor_scalar_min(f[:], r10[:], 1.0)
            if first:
                nc.vector.tensor_tensor(cs2_new[:, sl], f[:], f[:], ALU.mult)
            else:
                ff = stile_s()
                nc.vector.tensor_tensor(ff[:], f[:], f[:], ALU.mult)
                nc.vector.tensor_tensor(cs2_new[:, sl], S["cs2"][:, sl],
                                        ff[:], ALU.mult)
        S["cs2"] = cs2_new

    # ---------- Layers 1, 2 ----------
    t1p = tc.alloc_tile_pool(name="t1", bufs=1, side="left")
    t1, fin1 = gemm_layer(t0, C["w1"], C["kt1"], C["mt1"], t1p, "t1_",
                          preloaded=w1_pre)
    t0p.release()

    t2p = tc.alloc_tile_pool(name="t2", bufs=1, side="right")
    t2, fin2 = gemm_layer(t1, C["w2"], C["kt2"], C["mt2"], t2p, "t2_",
                          mid_fn=lambda: boundary(fin1, first=True))
    t1p.release()

    # ---------- Layer 3 + expmap0/projx ----------
    # Chunk 0 is full-width (512); its tail hides inside the paired pass.
    # The last two chunks are half-width and PAIRED: one m-loop, one weight
    # load per m-tile feeding both psum chunks (so weight DMA bandwidth is
    # identical to a 512 chunk), with a fused two-chunk tail at the end -
    # Ln calls batched before Exp calls so the act-table set switches once.
    kt, mt = C["kt3"], C["mt3"]
    csp = tc.alloc_tile_pool(name="csp", bufs=1)
    tailp = tc.alloc_tile_pool(name="tailp", bufs=12)  # [1,w] chain tiles
    bcast = tc.alloc_tile_pool(name="bcast", bufs=2)
    outp = tc.alloc_tile_pool(name="outp", bufs=12)
    y3p = tc.alloc_tile_pool(name="y3", bufs=1, side="left")
    y3 = [y3p.tile([P, tokpc], BF16, tag=f"y3_{m}", name=f"y3_{m}")
          for m in range(mt)]

    def stile_w(w):
        return tailp.tile([1, w], F32, tag="sw", name="sw")

    def tail_head(sl, w, acc3, ci):
        """norm reduce + Ln for one chunk; returns state for tail_rest."""
        ps_norm = psn.tile([1, w], F32, tag="psn", name=f"psn3_{ci}")
        nc.tensor.matmul(ps_norm[:], ones_f[:], acc3[:],
                         start=True, stop=True)
        m2 = stile_w(w)
        nc.vector.tensor_tensor(m2[:], ps_norm[:], S["cs2"][:, sl], ALU.mult)
        Lm = stile_w(w)
        nc.scalar.activation(Lm[:], m2[:], AF.Ln, bias=c_eps2[0:1, :])
        return (sl, w, m2, Lm)

    def tail_chain(*states):
        """exp chain + broadcast for 1+ chunks, interleaved stepwise so the
        scalar and DVE chains of the chunks pipeline against each other.
        Returns [(sl, w, s3b, cosh_c), ...]."""
        invs, ns, ncls, ics, es, ens, shs, s3s, res = \
            [], [], [], [], [], [], [], [], []
        for (sl, w, m2, Lm) in states:
            inv_n = stile_w(w)
            nc.scalar.activation(inv_n[:], Lm[:], AF.Exp, scale=-0.5)
            invs.append(inv_n)
        for (sl, w, m2, Lm), inv_n in zip(states, invs):
            n_ = stile_w(w)
            nc.vector.tensor_tensor(n_[:], m2[:], inv_n[:], ALU.mult)
            ns.append(n_)
        for (sl, w, m2, Lm), n_ in zip(states, ns):
            ncl = stile_w(w)
            nc.vector.tensor_scalar_min(ncl[:], n_[:], MAX_TAN_NORM)
            ncls.append(ncl)
        for (sl, w, m2, Lm), inv_n in zip(states, invs):
            ic = stile_w(w)
            nc.vector.tensor_tensor(ic[:], inv_n[:], S["cs"][:, sl],
                                    ALU.mult)
            ics.append(ic)
        for (sl, w, m2, Lm), ncl in zip(states, ncls):
            e_ = stile_w(w)
            nc.scalar.activation(e_[:], ncl[:], AF.Exp, scale=1.0,
                                 bias=c_lnhalf[0:1, :])
            en = stile_w(w)
            nc.scalar.activation(en[:], ncl[:], AF.Exp, scale=-1.0,
                                 bias=c_lnhalf[0:1, :])
            es.append(e_)
            ens.append(en)
        for (sl, w, m2, Lm), e_, en, ic in zip(states, es, ens, ics):
            sh = stile_w(w)
            nc.vector.tensor_tensor(sh[:], e_[:], en[:], ALU.subtract)
            s3 = bcast.tile([1, w], BF16, tag="s3s", name="s3")
            nc.vector.tensor_tensor(s3[:], sh[:], ic[:], ALU.mult)
            s3b = bcast.tile([P, w], BF16, tag="s3b", name="s3b")
            nc.gpsimd.partition_broadcast(s3b[:], s3[:])
            s3s.append(s3b)
        for (sl, w, m2, Lm), e_, en, s3b in zip(states, es, ens, s3s):
            cosh_c = stile_w(w)
            nc.vector.tensor_tensor(cosh_c[:], e_[:], en[:], ALU.add)
            res.append((sl, w, s3b, cosh_c))
        return res

    def tail_mults(bstate, dma_eng=None):
        sl, w, s3b, cosh_c = bstate
        dma_eng = dma_eng or nc.gpsimd
        for m in range(mt):
            ot = outp.tile([P, w], F32, tag="ot", name="ot")
            nc.vector.tensor_tensor(ot[:], y3[m][:, sl], s3b[:], ALU.mult)
            if m == 0:
                nc.vector.tensor_copy(ot[0:1, :], cosh_c[:])
            dma_eng.dma_start(C["out"].ap()[m * P:(m + 1) * P, sl], ot[:])

    def eviction(ps, acc3, m, sl, w):
        if m == 0:
            nc.scalar.activation(acc3[:], ps[:], AF.Square)
            nc.vector.memset(acc3[0:1, :], 0.0)
        else:
            sq = sqp.tile([P, w], F32, tag="sq", name="sq")
            nc.scalar.activation(sq[:], ps[:], AF.Square)
            nc.vector.tensor_tensor(acc3[:], acc3[:], sq[:], ALU.add)
        nc.scalar.activation(y3[m][:, sl], ps[:], AF.Identity)

    # --- pass 0: chunk [0:ch] solo ---
    sl0 = slice(0, ch)
    acc3_0 = accp.tile([P, ch], F32, tag="acc", name="acc3_0")
    for m in range(mt):
        wm = wp.tile([P, kt * P], BF16, tag="wtile", name="wm")
        nc.sync.dma_start(wm[:], C["w3"].ap()[m * P:(m + 1) * P, :])
        ps = psy.tile([P, ch], F32, tag="psy", name="psy3")
        for k in range(kt):
            nc.tensor.matmul(ps[:], wm[:, k * P:(k + 1) * P], t2[k][:, sl0],
                             start=(k == 0), stop=(k == kt - 1))
        if m == 1:
            boundary(fin2, first=False)
            # cs = sqrt(cs2) via exp(0.5*ln(.)), off the critical path -
            # the final tails then need only ONE Ln each
            cs = csp.tile([1, tokpc], F32, tag="cs", name="cs")
            for cc in range(nch):
                ssl = slice(cc * ch, (cc + 1) * ch)
                Lc = stile_s()
                nc.scalar.activation(Lc[:], S["cs2"][:, ssl], AF.Ln)
                nc.scalar.activation(cs[:, ssl], Lc[:], AF.Exp, scale=0.5)
            S["cs"] = cs
        eviction(ps, acc3_0, m, sl0, ch)

    # --- pass 1: chunks [ch:ch+hw] and [ch+hw:tokpc] paired ---
    hw = (tokpc - ch) // 2
    slA = slice(ch, ch + hw)
    slB = slice(ch + hw, tokpc)
    accA = accp.tile([P, hw], F32, tag="acc", name="acc3_A")
    accB = accp.tile([P, hw], F32, tag="acc", name="acc3_B")
    tail0_done = False
    for m in range(mt):
        wm = wp.tile([P, kt * P], BF16, tag="wtile", name="wm")
        nc.sync.dma_start(wm[:], C["w3"].ap()[m * P:(m + 1) * P, :])
        psA = psy.tile([P, hw], F32, tag="psy", name="psy3A")
        psB = psy.tile([P, hw], F32, tag="psy", name="psy3B")
        for k in range(kt):
            nc.tensor.matmul(psA[:], wm[:, k * P:(k + 1) * P], t2[k][:, slA],
                             start=(k == 0), stop=(k == kt - 1))
            nc.tensor.matmul(psB[:], wm[:, k * P:(k + 1) * P], t2[k][:, slB],
                             start=(k == 0), stop=(k == kt - 1))
        if m == 1 and not tail0_done:
            st0 = tail_head(sl0, ch, acc3_0, 0)
            tail_mults(tail_chain(st0)[0])
            tail0_done = True
        eviction(psA, accA, m, slA, hw)
        eviction(psB, accB, m, slB, hw)

    stA = tail_head(slA, hw, accA, 1)
    stB = tail_head(slB, hw, accB, 2)
    bsA, bsB = tail_chain(stA, stB)
    # A's output DMAs issue on the GpSimd queue, B's on Sync (idle by now)
    # so the 16 half-width descriptors don't serialize on one queue
    for m in range(mt):
        otA = outp.tile([P, hw], F32, tag="ot", name="otA")
        nc.vector.tensor_tensor(otA[:], y3[m][:, bsA[0]], bsA[2][:],
                                ALU.mult)
        if m == 0:
            nc.vector.tensor_copy(otA[0:1, :], bsA[3][:])
        nc.gpsimd.dma_start(C["out"].ap()[m * P:(m + 1) * P, bsA[0]],
                            otA[:])
        otB = outp.tile([P, hw], F32, tag="ot", name="otB")
        nc.vector.tensor_tensor(otB[:], y3[m][:, bsB[0]], bsB[2][:],
                                ALU.mult)
        if m == 0:
            nc.vector.tensor_copy(otB[0:1, :], bsB[3][:])
        nc.sync.dma_start(C["out"].ap()[m * P:(m + 1) * P, bsB[0]],
                          otB[:])
    y3p.release()
    outp.release()
    bcast.release()
    tailp.release()
    csp.release()
    t2p.release()

    for p in (psn, psy, wp, accbp, accp, sqbp, sqp, scalS, scalL, const):
        p.release()


# =====================================================================
# General (nonzero-bias) fallback program - barrier between layers
# =====================================================================

def build_nc_general(tokpc=TOKPC, din=D_IN, dhid=D_HID, dout=D_OUT, ch=512):
    assert tokpc % ch == 0
    nch = tokpc // ch
    kt1, mt1 = din // P, dhid // P
    kt2, mt2 = dhid // P, dhid // P
    kt3, mt3 = dhid // P, dout // P

    nc = bacc.Bacc("TRN2", target_bir_lowering=False, debug=False,
                   num_devices=N_CORES)

    xt_d = nc.dram_tensor("xt", [din, tokpc], BF16, kind="ExternalInput")
    x0_d = nc.dram_tensor("x0", [1, tokpc], F32, kind="ExternalInput")
    w1_d = nc.dram_tensor("w1", [mt1 * P, din], BF16, kind="ExternalInput")
    w2_d = nc.dram_tensor("w2", [mt2 * P, dhid], BF16, kind="ExternalInput")
    w3_d = nc.dram_tensor("w3", [mt3 * P, dhid], BF16, kind="ExternalInput")
    b1_d = nc.dram_tensor("b1", [P, mt1], F32, kind="ExternalInput")
    b2_d = nc.dram_tensor("b2", [P, mt2], F32, kind="ExternalInput")
    b3_d = nc.dram_tensor("b3", [P, mt3], F32, kind="ExternalInput")
    out_d = nc.dram_tensor("out", [dout, tokpc], F32, kind="ExternalOutput")

    with tile.TileContext(nc) as tc:
        _build_general_program(tc, nc, dict(
            tokpc=tokpc, din=din, dhid=dhid, dout=dout, ch=ch, nch=nch,
            kt1=kt1, mt1=mt1, kt2=kt2, mt2=mt2, kt3=kt3, mt3=mt3,
            xt=xt_d, x0=x0_d, w1=w1_d, w2=w2_d, w3=w3_d,
            b1=b1_d, b2=b2_d, b3=b3_d, out=out_d,
        ))
    nc.compile()
    return nc


def _build_general_program(tc, nc, C):
    tokpc, ch, nch = C["tokpc"], C["ch"], C["nch"]

    const = tc.alloc_tile_pool(name="const", bufs=1)
    scalL = tc.alloc_tile_pool(name="scalL", bufs=4)
    scalS = tc.alloc_tile_pool(name="scalS", bufs=5)
    bcast = tc.alloc_tile_pool(name="bcast", bufs=2)
    sqp = tc.alloc_tile_pool(name="sq", bufs=2)
    accp = tc.alloc_tile_pool(name="acc", bufs=4)
    wp = tc.alloc_tile_pool(name="wt", bufs=2)
    psy = tc.alloc_tile_pool(name="psy", bufs=4, space="PSUM")
    psn = tc.alloc_tile_pool(name="psn", bufs=4, space="PSUM")
    outp = tc.alloc_tile_pool(name="outp", bufs=3)

    bias1 = const.tile([P, C["mt1"]], F32, tag="bias1")
    nc.sync.dma_start(bias1[:], C["b1"].ap())
    bias2 = const.tile([P, C["mt2"]], F32, tag="bias2")
    nc.sync.dma_start(bias2[:], C["b2"].ap())
    bias3 = const.tile([P, C["mt3"]], F32, tag="bias3")
    nc.sync.dma_start(bias3[:], C["b3"].ap())
    ones_f = const.tile([P, 1], F32, tag="ones_f", name="ones_f")
    nc.vector.memset(ones_f[:], 1.0)
    c_eps2 = const.tile([P, 1], F32, tag="c_eps2", name="c_eps2")
    nc.vector.memset(c_eps2[:], EPS2)
    c_ln10 = const.tile([P, 1], F32, tag="c_ln10", name="c_ln10")
    nc.vector.memset(c_ln10[:], LN10)
    c_lnhalf = const.tile([P, 1], F32, tag="c_lnhalf", name="c_lnhalf")
    nc.vector.memset(c_lnhalf[:], LNHALF)

    def stile_l():
        return scalL.tile([1, tokpc], F32, tag="sl", name="sl")

    def stile_s():
        return scalS.tile([1, ch], F32, tag="ss", name="ss")

    def norm_accum_tiles():
        return [psn.tile([1, ch], F32, tag="psn", name=f"psn{_}")
                for _ in range(nch)]

    def bcast_full(s_full):
        sb = bcast.tile([P, tokpc], F32, tag="sb", name="sb")
        nc.gpsimd.partition_broadcast(sb[:], s_full[:])
        return sb

    def gemm_layer(tin, w_d, bias_t, kt, mt, out_pool, out_dtype, tag):
        accs = [accp.tile([P, ch], F32, tag="acc", name=f"acc{_}")
                for _ in range(nch)]
        tout = []
        for m in range(mt):
            wm = wp.tile([P, kt * P], BF16, tag="wtile", name="wm")
            nc.sync.dma_start(wm[:], w_d.ap()[m * P:(m + 1) * P, :])
            pss = [psy.tile([P, ch], F32, tag="psy", name=f"psy{_}")
                   for _ in range(nch)]
            for k in range(kt):
                for c in range(nch):
                    nc.tensor.matmul(pss[c][:], wm[:, k * P:(k + 1) * P],
                                     tin[k][:, c * ch:(c + 1) * ch],
                                     start=(k == 0), stop=(k == kt - 1))
            ty = out_pool.tile([P, tokpc], out_dtype, tag=f"{tag}{m}",
                               name=f"{tag}{m}")
            for c in range(nch):
                nc.scalar.activation(ty[:, c * ch:(c + 1) * ch], pss[c][:],
                                     AF.Identity, bias=bias_t[:, m:m + 1],
                                     scale=1.0)
                if m == 0:
                    nc.scalar.activation(accs[c][:], pss[c][:], AF.Square,
                                         bias=bias_t[:, m:m + 1], scale=1.0)
                    nc.vector.memset(accs[c][0:1, :], 0.0)
                else:
                    sq = sqp.tile([P, ch], F32, tag="sq", name="sq")
                    nc.scalar.activation(sq[:], pss[c][:], AF.Square,
                                         bias=bias_t[:, m:m + 1], scale=1.0)
                    nc.vector.tensor_tensor(accs[c][:], accs[c][:], sq[:],
                                            ALU.add)
            if m == 0:
                nc.vector.memset(ty[0:1, :], 0.0)
            tout.append(ty)

        def finish():
            ps_norm = norm_accum_tiles()
            for c in range(nch):
                nc.tensor.matmul(ps_norm[c][:], ones_f[:], accs[c][:],
                                 start=True, stop=True)
            return ps_norm
        return tout, finish

    def clamp_scale(ps_norm):
        """s = min(max(sqrt(ssq),eps),10)/max(sqrt(ssq),eps) via ln/exp."""
        s = stile_l()
        for c in range(nch):
            L = stile_s()
            nc.scalar.activation(L[:], ps_norm[c][:], AF.Ln, bias=c_eps2[0:1, :])
            r10 = stile_s()
            nc.scalar.activation(r10[:], L[:], AF.Exp, scale=-0.5,
                                 bias=c_ln10[0:1, :])
            f = stile_s()
            nc.vector.tensor_scalar_min(f[:], r10[:], 1.0)
            nc.vector.tensor_copy(s[:, c * ch:(c + 1) * ch], f[:])
        return s

    def apply_scale(tiles, sb):
        for t in tiles:
            nc.vector.tensor_tensor(t[:], t[:], sb[:], ALU.mult)

    def body():
        # Phase 0: load bf16 xs (= raw t0), input norm, s0
        t0p = tc.alloc_tile_pool(name="t0", bufs=1, side="right")
        t0 = []
        for k in range(C["kt1"]):
            t = t0p.tile([P, tokpc], BF16, tag=f"t0_{k}", name=f"t0_{k}")
            nc.sync.dma_start(t[:], C["xt"].ap()[k * P:(k + 1) * P, :])
            t0.append(t)
        x0t = stile_l()
        nc.sync.dma_start(x0t[:], C["x0"].ap())

        acc0 = [accp.tile([P, ch], F32, tag="acc", name=f"acc0_{_}")
                for _ in range(nch)]
        for k in range(C["kt1"]):
            for c in range(nch):
                if k == 0:
                    nc.scalar.activation(acc0[c][:],
                                         t0[k][:, c * ch:(c + 1) * ch],
                                         AF.Square)
                else:
                    sq = sqp.tile([P, ch], F32, tag="sq", name="sq")
                    nc.scalar.activation(sq[:], t0[k][:, c * ch:(c + 1) * ch],
                                         AF.Square)
                    nc.vector.tensor_tensor(acc0[c][:], acc0[c][:], sq[:],
                                            ALU.add)

        # norm-MM for the input + s0 = arccosh(x0) / ||xs|| chain
        # (x0 input here is pre-clipped arccosh distance d, computed on host)
        ps_n0 = norm_accum_tiles()
        for c in range(nch):
            nc.tensor.matmul(ps_n0[c][:], ones_f[:], acc0[c][:],
                             start=True, stop=True)
        s0 = stile_l()
        for c in range(nch):
            L = stile_s()
            nc.scalar.activation(L[:], ps_n0[c][:], AF.Ln, bias=c_eps2[0:1, :])
            r = stile_s()
            nc.scalar.activation(r[:], L[:], AF.Exp, scale=-0.5)
            nc.vector.tensor_tensor(s0[:, c * ch:(c + 1) * ch],
                                    x0t[:, c * ch:(c + 1) * ch], r[:],
                                    ALU.mult)

        s0b = bcast_full(s0)
        for k in range(C["kt1"]):
            nc.vector.tensor_tensor(t0[k][:], t0[k][:], s0b[:], ALU.mult)

        # Layers 1, 2 with barrier scale application
        t1p = tc.alloc_tile_pool(name="t1", bufs=1, side="left")
        t1, fin1 = gemm_layer(t0, C["w1"], bias1, C["kt1"], C["mt1"],
                              t1p, BF16, "t1_")
        apply_scale(t1, bcast_full(clamp_scale(fin1())))
        t0p.release()

        t2p = tc.alloc_tile_pool(name="t2", bufs=1, side="right")
        t2, fin2 = gemm_layer(t1, C["w2"], bias2, C["kt2"], C["mt2"],
                              t2p, BF16, "t2_")
        apply_scale(t2, bcast_full(clamp_scale(fin2())))
        t1p.release()

        # Layer 3 + expmap0/projx
        kt, mt = C["kt3"], C["mt3"]
        y3p = tc.alloc_tile_pool(name="y3", bufs=1, side="left")
        y3 = [y3p.tile([P, tokpc], F32, tag=f"y3_{m}", name=f"y3_{m}")
              for m in range(mt)]
        deferred_tail = None
        for c in range(nch):
            sl = slice(c * ch, (c + 1) * ch)
            acc3 = accp.tile([P, ch], F32, tag="acc", name=f"acc3_{c}")
            for m in range(mt):
                wm = wp.tile([P, kt * P], BF16, tag="wtile", name="wm")
                nc.sync.dma_start(wm[:], C["w3"].ap()[m * P:(m + 1) * P, :])
                ps = psy.tile([P, ch], F32, tag="psy", name="psy3")
                for k in range(kt):
                    nc.tensor.matmul(ps[:], wm[:, k * P:(k + 1) * P],
                                     t2[k][:, sl],
                                     start=(k == 0), stop=(k == kt - 1))
                if m == 1 and deferred_tail is not None:
                    deferred_tail()
                    deferred_tail = None
                nc.scalar.activation(y3[m][:, sl], ps[:], AF.Identity,
                                     bias=bias3[:, m:m + 1], scale=1.0)
                if m == 0:
                    nc.scalar.activation(acc3[:], ps[:], AF.Square,
                                         bias=bias3[:, m:m + 1], scale=1.0)
                    nc.vector.memset(acc3[0:1, :], 0.0)
                else:
                    sq = sqp.tile([P, ch], F32, tag="sq", name="sq")
                    nc.scalar.activation(sq[:], ps[:], AF.Square,
                                         bias=bias3[:, m:m + 1], scale=1.0)
                    nc.vector.tensor_tensor(acc3[:], acc3[:], sq[:], ALU.add)

            def chunk_tail(sl=sl, acc3=acc3, c=c):
                ps_norm = psn.tile([1, ch], F32, tag="psn", name=f"psn3_{c}")
                nc.tensor.matmul(ps_norm[:], ones_f[:], acc3[:],
                                 start=True, stop=True)
                Lr = stile_s()
                nc.scalar.activation(Lr[:], ps_norm[:], AF.Ln, bias=c_eps2[0:1, :])
                rr = stile_s()
                nc.scalar.activation(rr[:], Lr[:], AF.Exp, scale=-0.5)
                n_ = stile_s()
                nc.scalar.activation(n_[:], Lr[:], AF.Exp, scale=0.5)
                ncl = stile_s()
                nc.vector.tensor_scalar_min(ncl[:], n_[:], MAX_TAN_NORM)
                e_ = stile_s()
                nc.scalar.activation(e_[:], ncl[:], AF.Exp, scale=1.0,
                                     bias=c_lnhalf[0:1, :])
                en = stile_s()
                nc.scalar.activation(en[:], ncl[:], AF.Exp, scale=-1.0,
                                     bias=c_lnhalf[0:1, :])
                cosh_c = stile_s()
                nc.vector.tensor_tensor(cosh_c[:], e_[:], en[:], ALU.add)
                sh = stile_s()
                nc.vector.tensor_tensor(sh[:], e_[:], en[:], ALU.subtract)
                s3 = stile_s()
                nc.vector.tensor_tensor(s3[:], sh[:], rr[:], ALU.mult)
                s3b = bcast.tile([P, ch], F32, tag="s3b", name="s3b")
                nc.gpsimd.partition_broadcast(s3b[:], s3[:])
                for m in range(mt):
                    ot = outp.tile([P, ch], F32, tag="ot", name="ot")
                    nc.vector.tensor_tensor(ot[:], y3[m][:, sl], s3b[:],
                                            ALU.mult)
                    if m == 0:
                        nc.vector.tensor_copy(ot[0:1, :], cosh_c[:])
                    nc.sync.dma_start(C["out"].ap()[m * P:(m + 1) * P, sl],
                                      ot[:])

            deferred_tail = chunk_tail
        deferred_tail()
        t2p.release()
        y3p.release()

    body()
    for p in (outp, psn, psy, wp, accp, sqp, bcast, scalS, scalL, const):
        p.release()


# =====================================================================
# host-side prep + entry point
# =====================================================================

def _block_weight(w):
    """W [dout, din] f32 -> [mt*128, din] bf16 with row m*128+p holding, for
    each k-tile, lhsT tile (k,m) row p: out[m*128+p, k*128+j] = W.T[k*128+p,
    m*128+j].  One fully-contiguous [128, kt*128] DMA per m-tile."""
    dout, din = w.shape
    mt, kt = dout // P, din // P
    w = np.asarray(w, dtype=np.float32)
    blocked = (w.reshape(mt, P, kt, P)       # [m, j, k, p]
                .transpose(0, 3, 2, 1)       # [m, p, k, j]
                .reshape(mt * P, din))
    return np.ascontiguousarray(blocked.astype(ml_dtypes.bfloat16))


def _prep_bias(b, mt):
    """b [d] -> [128, mt] f32 with out[p, m] = b[m*128+p]."""
    return np.ascontiguousarray(
        np.asarray(b, dtype=np.float32).reshape(mt, P).T)


@functools.lru_cache(maxsize=2)
def _get_nc(fold=True):
    return build_nc_fold() if fold else build_nc_general()


def prep_in_maps_fold(x_hyp, W1, W2, W3):
    """logmap0 on the host: t0 = [0, d*xs/||xs||] feature-major bf16."""
    w1b = _block_weight(W1)
    w2b = _block_weight(W2)
    w3b = _block_weight(W3)
    x = np.asarray(x_hyp, dtype=np.float32)
    d_all = np.arccosh(np.maximum(x[:, 0], 1.0 + EPS))          # [TOK]
    xs_all = x[:, 1:]                                           # [TOK, 1023]
    ns_all = np.maximum(np.linalg.norm(xs_all, axis=1), EPS)
    s0_all = (d_all / ns_all).astype(np.float32)                # [TOK]
    in_maps = []
    for c in range(N_CORES):
        rows = slice(c * TOKPC, (c + 1) * TOKPC)
        t0 = np.zeros((D_IN, TOKPC), dtype=np.float32)
        t0[1:, :] = (xs_all[rows] * s0_all[rows, None]).T
        xt = t0.astype(ml_dtypes.bfloat16)
        in_maps.append(dict(xt=np.ascontiguousarray(xt),
                            w1=w1b, w2=w2b, w3=w3b))
    return in_maps


def prep_in_maps_general(x_hyp, W1, b1, W2, b2, W3, b3):
    w1b = _block_weight(W1)
    w2b = _block_weight(W2)
    w3b = _block_weight(W3)
    b1c = _prep_bias(b1, D_HID // P)
    b2c = _prep_bias(b2, D_HID // P)
    b3c = _prep_bias(b3, D_OUT // P)
    x = np.asarray(x_hyp, dtype=np.float32)
    in_maps = []
    for c in range(N_CORES):
        shard = x[c * TOKPC:(c + 1) * TOKPC, :]  # [tokpc, din]
        xt = shard.T.astype(ml_dtypes.bfloat16)
        xt[0, :] = 0  # zero time component (norm + GEMM both want it out)
        # x0 slot carries d = arccosh(clip(x0)) precomputed on host
        d = np.arccosh(np.maximum(shard[:, 0:1], 1.0 + EPS)).T
        in_maps.append(dict(xt=np.ascontiguousarray(xt),
                            x0=np.ascontiguousarray(d.astype(np.float32)),
                            w1=w1b, w2=w2b, w3=w3b,
                            b1=b1c, b2=b2c, b3=b3c))
    return in_maps


LAST_RESULTS = None


def kernel(x_hyp, W1, b1, W2, b2, W3, b3):
    global LAST_RESULTS
    fold = not (np.any(b1) or np.any(b2) or np.any(b3))
    nc = _get_nc(fold)
    if fold:
        in_maps = prep_in_maps_fold(x_hyp, W1, W2, W3)
    else:
        in_maps = prep_in_maps_general(x_hyp, W1, b1, W2, b2, W3, b3)
    res = bass_utils.run_bass_kernel_spmd(nc, in_maps,
                                          core_ids=list(range(N_CORES)))
    LAST_RESULTS = res
    parts = [np.asarray(res.results[c]["out"]).T for c in range(N_CORES)]
    return np.ascontiguousarray(np.concatenate(parts, axis=0),
                                dtype=np.float32)
